# revision 58
# baseline (speedup 1.0000x reference)
"""Causal self-attention (d_model=1024, n_head=16, seq=4096) on 8 trn2 cores.

Sharding: tensor-parallel over heads (2 heads/core) for QKV + attention.
The output re-shard uses EIGHT small AllToAlls (one per 512-row q-block)
over an INTERLEAVED row partition -- core j owns rows {512*n + 64*j + i} --
so each collective fires right after its q-block's softmax-normalize and
overlaps the remaining attention.  The received rows' output projection is
braided into late attention blocks as PE filler; only the last q-block's
collective + projection remain in the serial tail.  The host reorders the
interleaved row shards.

Per-core layout (bf16 into the PE, fp32 PSUM accumulation):
  - x^T built via PE identity-matmul transposes (the d_model contraction
    needs x in [c, T] layout for both qkv operands).
  - qkv^T = w_slice.T @ x^T lands directly in [chan, T] layout, so qT/kT
    are exactly the lhsT/rhs of the score matmul (scores^T = K Q^T), and
    V' (normal orientation + a ones column per head) comes from small PE
    transposes.
  - softmax without max-subtraction (scores ~ N(0,1): exp cannot overflow
    fp32).  The PV matmul runs in [q, d] orientation (lhsT = P^T tile,
    rhs = V'), so each accumulation step moves only N=65 output columns
    (matmul cost is out-free-size cycles) -- half the PE cycles of the
    [d, q] orientation -- and the softmax denominator lands as a
    per-partition scalar, normalized with reciprocal + tensor_scalar
    multiplies (no cross-partition broadcast needed).  One PSUM
    accumulation group per bank: start marks the whole 2KB zero-region
    pending-zero; the four 128-row q-subtiles share it legally.
  - receivers transpose the [q, c] rows back to the projection's lhsT
    layout with small PE identity matmuls.
  - causal masking: only lower-triangle k-tiles are computed; diagonal
    tiles are masked by a precomputed 0/1 multiply after the exp.
  - emission is braided: prep for block n+1 (x load/transpose/qkv/V') and
    late projection pieces are interleaved INSIDE the attention groups of
    q-block n (between the score matmuls and the exp/mask/PV chain), so
    the PE stream has work while ACT runs the exps.  PSUM->SBUF copies run
    on DVE; x bf16 casts on gpsimd; exp is the only ACT-engine work.
  - collective n is issued one block late (after block n+1's x casts) so
    its wait never head-of-line blocks the Pool queue.
"""

import sys
import types

import numpy as np
import ml_dtypes

D_MODEL = 1024
N_HEAD = 16
SEQ = 4096
N_CORES = 8
D_HEAD = 64
CPC = 128            # channels per core (2 heads x 64)
QB = 512             # attention q-block width
BF16 = ml_dtypes.bfloat16
XBAR_FROM_BLOCK = 99   # blocks >= this: x^T via ACT-queue xbar into scratch
PBC_NORM = False       # normalize broadcast via gpsimd partition_broadcast


def _install_compat_patches():
    """Stub antenv.axon_hooks (absent in this container) so
    run_bass_kernel_spmd's trace path degrades instead of ImportError."""
    if "antenv.axon_hooks" not in sys.modules:
        mod = types.ModuleType("antenv.axon_hooks")
        mod.get_axon_ntff_profile_hook = lambda: None
        sys.modules["antenv.axon_hooks"] = mod


def _split_multi_waits(nc):
    """The nix walrus here accepts at most ONE sync-wait per instruction
    (setupSyncWait: 'Too many sync wait commands').  Hoist extra waits onto
    same-engine NoOps inserted immediately before the instruction — engine
    streams execute in program order, so semantics are unchanged."""
    import concourse.mybir as mybir

    n = 0
    for fn in nc.m.functions:
        for bb in fn.blocks:
            insts = bb.instructions
            out = []
            for inst in insts:
                si = getattr(inst, "sync_info", None)
                waits = list(si.on_wait) if si is not None else []
                if len(waits) > 1:
                    si.on_wait.clear()
                    for w in waits[:-1]:
                        n += 1
                        nop = mybir.InstNoOp(name=f"I-WSPLIT{n}", ins=[], outs=[])
                        nop.engine = inst.engine
                        nop.sync_info = mybir.SyncInfo(on_wait=[w], on_update=[])
                        out.append(nop)
                    si.on_wait.append(waits[-1])
                out.append(inst)
            bb.instructions = out


def build_nc(seq=SEQ, use_collective=True, split_waits=True):
    """Build the single-core SPMD program (identical on all 8 cores)."""
    import concourse.bass as bass
    import concourse.mybir as mybir
    from concourse.tile import TileContext

    _install_compat_patches()

    f32 = mybir.dt.float32
    bf16 = mybir.dt.bfloat16
    AFT = mybir.ActivationFunctionType

    from concourse import library_config

    nT = seq // 128       # T-tiles
    nQB = seq // QB       # attention q-blocks
    SW = seq // N_CORES   # AllToAll shard width (output rows per core)

    nc = bass.Bass("TRN2", target_bir_lowering=False, debug=False,
                   num_devices=N_CORES)
    x_d = nc.dram_tensor("x", [seq, D_MODEL], f32, kind="ExternalInput").ap()
    wq_d = nc.dram_tensor("w_slice", [D_MODEL, 3 * CPC], f32,
                          kind="ExternalInput").ap()
    wp_d = nc.dram_tensor("w_proj", [D_MODEL, D_MODEL], f32,
                          kind="ExternalInput").ap()
    id_d = nc.dram_tensor("ident", [128, 128], bf16, kind="ExternalInput").ap()
    mk_d = nc.dram_tensor("masks", [4, 128, QB], bf16,
                          kind="ExternalInput").ap()
    out_d = nc.dram_tensor("out", [SW, D_MODEL], f32,
                           kind="ExternalOutput").ap()

    with TileContext(nc) as tc:
        with (
            tc.tile_pool(name="per", bufs=1) as per,
            tc.tile_pool(name="stg", bufs=2) as stg,
            tc.tile_pool(name="dram", bufs=1, space="DRAM") as dram,
        ):
            qT = per.tile([128, seq], bf16)      # [2 heads x 64 d, T]
            kT = per.tile([128, seq], bf16)
            Vp = per.tile([128, nT, 130], bf16)  # V' tiles: [v_h0|1|v_h1|1]
            wqkv = per.tile([128, 8, 3 * CPC], bf16)
            wpj = per.tile([128, 8, D_MODEL], bf16)
            iden = per.tile([128, 128], bf16)
            mks = per.tile([128, 4, QB], bf16)
            ones = per.tile([128, 64], f32)

            nc.scalar.dma_start(iden[:], id_d[:])
            nc.any.memset(ones[:], 1.0)
            nc.any.memset(Vp[:, :, 64:65], 1.0)
            nc.any.memset(Vp[:, :, 129:130], 1.0)

            # (weight staging happens inside the xstg pool below)

            # per-q-block AllToAll buffers over the INTERLEAVED row shard:
            # core j owns rows {512*n + 64*j + i}; chunk j of a2a_in[n] is
            # this core's 2 heads of that 64-row slice, so collective n can
            # fire right after q-block n's normalize.
            a2a_in = [dram.tile([4, 2, 64, 128], bf16, name=f"a2ain_{n}")
                      for n in range(nQB)]
            a2a_out = [dram.tile([8, 64, 128], bf16, name=f"a2aout_{n}")
                       for n in range(nQB)]

            # ---- phases 0-2, braided emission ------------------------
            # Engines execute their scheduled streams in static order, so
            # overlap must be built into emission order: the prep work
            # (x-load/transpose/qkv/V') for block n+1 is interleaved chunk-
            # by-chunk between the attention groups of q-block n.  Attention
            # qb=n depends only on qkv blocks 0..n, so each braid is legal.
            # PSUM banks: pA 2x1 + sT 2x2 + yt0 1 + yt1 1 = 8
            with (
                tc.tile_pool(name="xp", bufs=1) as xp,
                tc.tile_pool(name="xstg", bufs=3) as xstg,
                tc.tile_pool(name="ps", bufs=2, space="PSUM") as ps,
            ):
                xT = xp.tile([128, 8, seq], bf16)   # [c-chunk part, chunk, T]

                def wqkv_stage():
                    for k in range(8):
                        wtmp = xstg.tile([128, 3 * CPC], f32, tag="wq",
                                         bufs=4, name=f"wtmp_{k}")
                        nc.sync.dma_start(wtmp[:],
                                          wq_d[128 * k:128 * (k + 1), :])
                        nc.vector.tensor_copy(wqkv[:, k, :], wtmp[:])

                def prep_chunks(n):
                    """Emit-closures for block n: loads, x^T xbar, qkv^T, V'."""
                    state = {}

                    def loads():
                        xbs = []
                        for u in range(4):
                            t = 4 * n + u
                            xf = xstg.tile([128, D_MODEL], f32, tag="xf",
                                           bufs=8, name=f"xf_{t}")
                            nc.sync.dma_start(xf[:],
                                              x_d[128 * t:128 * (t + 1), :])
                            xb = xstg.tile([128, D_MODEL], bf16, tag="xb",
                                           bufs=8, name=f"xb_{t}")
                            nc.gpsimd.tensor_copy(xb[:], xf[:])
                            xbs.append(xb)
                        state["xbs"] = xbs

                    def trans(j):
                        # j indexes (x-tile u = j//2, c-chunk quad a = j%2):
                        # one PSUM tile holds 4 c-chunk transposes of a
                        # single x-tile, so work starts after its one load
                        def emit():
                            u, a = divmod(j, 2)
                            tp = ps.tile([128, 512], f32, tag="pA",
                                         name=f"tp_{n}_{j}")
                            for c in range(4):
                                nc.tensor.matmul(
                                    tp[:, 128 * c:128 * (c + 1)],
                                    state["xbs"][u][:, 128 * (4 * a + c):
                                                    128 * (4 * a + c + 1)],
                                    iden[:], start=True, stop=True)
                            nc.vector.tensor_copy(
                                xT[:, 4 * a:4 * (a + 1),
                                   128 * (4 * n + u):
                                   128 * (4 * n + u + 1)],
                                tp[:])
                        return emit

                    def qkv(m):
                        def emit():
                            qp = ps.tile([128, 512], f32, tag="pA",
                                         name=f"qp_{n}_{m}")
                            for k in range(8):
                                nc.tensor.matmul(
                                    qp[:],
                                    wqkv[:, k, 128 * m:128 * (m + 1)],
                                    xT[:, k, 512 * n:512 * (n + 1)],
                                    start=(k == 0), stop=(k == 7))
                            if m == 0:
                                nc.vector.tensor_copy(
                                    qT[:, 512 * n:512 * (n + 1)], qp[:])
                            elif m == 1:
                                nc.vector.tensor_copy(
                                    kT[:, 512 * n:512 * (n + 1)], qp[:])
                            else:
                                vs = xstg.tile([128, 512], bf16, tag="vs",
                                               bufs=2, name=f"vs_{n}")
                                nc.vector.tensor_copy(vs[:], qp[:])
                                state["vs"] = vs
                        return emit

                    def vtr(u):
                        def emit():
                            t = 4 * n + u
                            vs = state["vs"]
                            # separate PSUM tiles per head: PE-write plus
                            # DVE-read of one PSUM bank is a HW fault
                            vp0 = ps.tile([128, 64], f32, tag="pA",
                                          name=f"vp0_{t}")
                            vp1 = ps.tile([128, 64], f32, tag="pA",
                                          name=f"vp1_{t}")
                            nc.tensor.matmul(
                                vp0[:], vs[0:64, 128 * u:128 * (u + 1)],
                                iden[0:64, 0:64], start=True, stop=True)
                            nc.tensor.matmul(
                                vp1[:], vs[64:128, 128 * u:128 * (u + 1)],
                                iden[64:128, 64:128], start=True, stop=True)
                            nc.vector.tensor_copy(Vp[:, t, 0:64], vp0[:])
                            nc.vector.tensor_copy(Vp[:, t, 65:129], vp1[:])
                        return emit

                    return ([loads] + [trans(j) for j in range(8)]
                            + [qkv(m) for m in range(3)]
                            + [vtr(u) for u in range(4)])

                carry = {}

                def off_of(qb, kt):
                    d = kt - 4 * qb
                    return 128 * d if d >= 0 else 0

                def emit_scores(qb, g):
                    # h-inner MM order: consecutive score matmuls use
                    # disjoint PE row-groups (h0 rows 0-63, h1 rows 64-127)
                    # so the 16x32x32-subarray PE overlaps them
                    sps = [ps.tile([128, 2 * QB], f32, tag="sT",
                                   name=f"sp_{qb}_{g}_{h}")
                           for h in (0, 1)]
                    for u in (0, 1):
                        kt = 2 * g + u
                        o = off_of(qb, kt)
                        for h in (0, 1):
                            nc.tensor.matmul(
                                sps[h][:, QB * u + o:QB * (u + 1)],
                                kT[64 * h:64 * (h + 1),
                                   128 * kt:128 * (kt + 1)],
                                qT[64 * h:64 * (h + 1),
                                   QB * qb + o:QB * (qb + 1)],
                                start=True, stop=True)
                    return sps

                def hoist_scores(qb):
                    # pre-emit the NEXT block's first score matmuls inside
                    # the current block's last-group filler, so the exp
                    # stream crosses the block boundary without a bubble
                    def emit():
                        carry[qb] = emit_scores(qb, 0)
                    return emit

                def attention_groups(qb, ytps, fill=None):
                    nkt = 4 * (qb + 1)

                    def group(g):
                        # diagonal k-tiles (d = kt-4qb >= 0) only attend to
                        # q >= 128d: trim score MM / exp / mask / yT MM to
                        # the valid column range [128d, QB).  q-cols below
                        # that are fully masked and, because kt=0 always
                        # covers the full width with start=True, never read.
                        def off(kt):
                            return off_of(qb, kt)

                        def emit():
                            if g == 0 and qb in carry:
                                sps = carry.pop(qb)
                            else:
                                sps = emit_scores(qb, g)
                            if fill is not None:
                                fill(g)
                            diag = off(2 * g) > 0 or off(2 * g + 1) > 0
                            for h in (0, 1):
                                pt = stg.tile([128, 2 * QB], bf16, tag="pT",
                                              bufs=3, name=f"pt_{qb}_{g}_{h}")
                                if diag:
                                    for u in (0, 1):
                                        o = off(2 * g + u)
                                        nc.scalar.activation(
                                            pt[:, QB * u + o:QB * (u + 1)],
                                            sps[h][:, QB * u + o:QB * (u + 1)],
                                            AFT.Exp, scale=0.125)
                                else:
                                    nc.scalar.activation(pt[:], sps[h][:],
                                                         AFT.Exp, scale=0.125)
                                for u in (0, 1):
                                    kt = 2 * g + u
                                    d = kt - 4 * qb
                                    o = off(kt)
                                    if d >= 0:
                                        w = min(o + 128, QB)
                                        nc.vector.tensor_mul(
                                            pt[:, QB * u + o:QB * u + w],
                                            pt[:, QB * u + o:QB * u + w],
                                            mks[:, d, o:w])
                                    # PV in [q, d] orientation: N=65 per
                                    # accumulation step instead of N=512
                                    # (matmul cost is out-free-size cycles).
                                    # One accumulation group per PSUM bank:
                                    # start marks the whole 2KB zero-region
                                    # pending-zero (first touch of each s
                                    # slot overwrites, later ones accumulate)
                                    for s in range(max(0, d), 4):
                                        nc.tensor.matmul(
                                            ytps[h][:, s, :],
                                            pt[:, QB * u + 128 * s:
                                               QB * u + 128 * (s + 1)],
                                            Vp[:, kt,
                                               65 * h:65 * (h + 1)],
                                            start=(kt == 0 and s == 0),
                                            stop=(kt == nkt - 1))
                        return emit

                    return [group(g) for g in range(nkt // 2)]

                def normalize(qb, ytps):
                    # y is [q, d] with q on partitions, so 1/denom is a
                    # per-partition scalar multiply -- no broadcast needed
                    ysn = stg.tile([128, 4, 128], bf16, tag="ysn", bufs=2,
                                   name=f"ysn_{qb}")
                    for h in (0, 1):
                        yq = stg.tile([128, 4, 66], f32, tag="dn", bufs=4,
                                      name=f"yq_{qb}_{h}")
                        nc.vector.tensor_copy(yq[:, :, 0:65], ytps[h][:])
                        nc.vector.reciprocal(yq[:, :, 65:66],
                                             yq[:, :, 64:65])
                        for s in range(4):
                            nc.vector.tensor_scalar_mul(
                                ysn[:, s, 64 * h:64 * (h + 1)],
                                yq[:, s, 0:64], yq[:, s, 65:66])
                    # stage this q-block's interleaved AllToAll rows:
                    # chunk j = (qsub j//2, partitions 64*(j%2)..) holds
                    # global rows {512*qb + 64*j + i} as [64 q, 128 c]
                    for jj in (0, 1):
                        nc.sync.dma_start(
                            a2a_in[qb][:, jj].rearrange("s p c -> p s c"),
                            ysn[64 * jj:64 * (jj + 1), :, :])

                def wpj_chunk(k):
                    def emit():
                        ptmp = xstg.tile([128, D_MODEL], f32, tag="wp",
                                         bufs=2, name=f"ptmp_{k}")
                        nc.sync.dma_start(ptmp[:],
                                          wp_d[128 * k:128 * (k + 1), :])
                        nc.gpsimd.tensor_copy(wpj[:, k, :], ptmp[:])
                    return emit

                def collective(qb):
                    def emit():
                        if use_collective:
                            nc.gpsimd.collective_compute(
                                "AllToAll", mybir.AluOpType.bypass,
                                ins=[a2a_in[qb].opt()],
                                outs=[a2a_out[qb].opt()],
                                replica_groups=[list(range(N_CORES))])
                        else:
                            # timing-model stand-in (TimelineSim can't
                            # execute collectives): DRAM->DRAM copy
                            nc.sync.dma_start(a2a_out[qb].opt(),
                                              a2a_in[qb].opt())
                    return emit

                def rx_piece(p):
                    """Projection for the 128 interleaved rows of q-blocks
                    2p, 2p+1 (this core's shard of those blocks); braided
                    into a late attention block whose collectives are done."""
                    st = {}

                    def rx_loads():
                        rxq = stg.tile([64, 2, 8, 128], bf16, tag="rxq",
                                       bufs=2, name=f"rxq_{p}")
                        for e in (0, 1):
                            nc.sync.dma_start(
                                rxq[:, e],
                                a2a_out[2 * p + e].rearrange("i p c -> p i c"))
                        st["rxq"] = rxq

                    def rtr():
                        # received rows are [q, c]; transpose per 128-c chunk
                        # into the projection's lhsT layout [c, q]
                        rxT = stg.tile([128, 8, 128], bf16, tag="rx",
                                       bufs=2, name=f"rxT_{p}")
                        for e in (0, 1):
                            ytp = ps.tile([128, 8, 64], f32, tag="pA",
                                          name=f"ytp_{p}_{e}")
                            for i in range(8):
                                nc.tensor.matmul(
                                    ytp[:, i, :], st["rxq"][:, e, i, :],
                                    iden[0:64, 0:64], start=True, stop=True)
                            nc.vector.tensor_copy(
                                rxT[:, :, 64 * e:64 * (e + 1)], ytp[:])
                        st["rx"] = rxT

                    def half(n2):
                        def emit():
                            pp = ps.tile([128, 512], f32, tag="pA",
                                         name=f"pp_{p}_{n2}")
                            for k in range(8):
                                nc.tensor.matmul(
                                    pp[:],
                                    st["rx"][:, k, :],
                                    wpj[:, k, 512 * n2:512 * (n2 + 1)],
                                    start=(k == 0), stop=(k == 7))
                            ob = stg.tile([128, 512], f32, tag="ob", bufs=2,
                                          name=f"ob_{p}_{n2}")
                            nc.vector.tensor_copy(ob[:], pp[:])
                            nc.sync.dma_start(
                                out_d[128 * p:128 * (p + 1),
                                      512 * n2:512 * (n2 + 1)], ob[:])
                        return emit

                    return [rx_loads, rtr, half(0), half(1)]

                p0 = prep_chunks(0)
                p0[0]()           # stage-0 x loads lead the DMA queues
                wqkv_stage()
                for m in range(4):
                    # masks load after the startup-critical x/w DMAs
                    nc.scalar.dma_start(mks[:, m, :], mk_d[m])
                for c in p0[1:]:
                    c()
                for n in range(nQB):
                    ytps = [ps.tile([128, 4, 65], f32, tag=f"yt{h}", bufs=1,
                                    name=f"yt{h}_{n}") for h in (0, 1)]
                    pend = prep_chunks(n + 1) if n + 1 < nQB else []
                    if n >= 1:
                        # issue q-block n-1's AllToAll one block late so its
                        # wait (on the staging DMAs) is already satisfied and
                        # never head-of-line blocks the Pool queue
                        pend = pend[:1] + [collective(n - 1)] + pend[1:]
                    if 3 <= n <= 4:
                        # stage w_proj in blocks 3-4: late enough to stay
                        # clear of the prep-limited early blocks, done
                        # before piece 0 consumes it at block 5
                        ks = {3: (0, 1, 2, 3), 4: (4, 5, 6, 7)}[n]
                        pend = pend + [wpj_chunk(k) for k in ks]
                    if n >= 5:
                        # piece p covers q-blocks 2p, 2p+1; its collectives
                        # completed >= 2 blocks ago
                        pend = pend + rx_piece(n - 5)
                    if n + 1 < nQB:
                        pend = pend + [hoist_scores(n + 1)]
                    st = {"ci": 0}

                    def fill(gi):
                        want = (gi + 1) * len(pend) // (2 * (n + 1))
                        while st["ci"] < want:
                            pend[st["ci"]]()
                            st["ci"] += 1

                    groups = attention_groups(n, ytps, fill)
                    for g in groups:
                        g()
                    while st["ci"] < len(pend):
                        pend[st["ci"]]()
                        st["ci"] += 1
                    normalize(n, ytps)
                collective(nQB - 1)()

            # ---- tail: projection piece for q-blocks 6,7 ------------------
            with tc.tile_pool(name="psC", bufs=1, space="PSUM") as psC:
                rxq = stg.tile([64, 2, 8, 128], bf16, tag="rxq", bufs=2,
                               name="rxq_3")
                for e in (0, 1):
                    nc.sync.dma_start(
                        rxq[:, e],
                        a2a_out[6 + e].rearrange("i p c -> p i c"))
                rx = stg.tile([128, 8, 128], bf16, tag="rx", bufs=2,
                              name="rxT_3")
                for e in (0, 1):
                    ytp = psC.tile([128, 8, 64], f32, tag="ytpC",
                                   name=f"ytpC_{e}")
                    for i in range(8):
                        nc.tensor.matmul(
                            ytp[:, i, :], rxq[:, e, i, :],
                            iden[0:64, 0:64], start=True, stop=True)
                    nc.vector.tensor_copy(
                        rx[:, :, 64 * e:64 * (e + 1)], ytp[:])
                for n2 in (0, 1):
                    pp = psC.tile([128, 512], f32, tag="ppC",
                                  name=f"ppC_{n2}")
                    for k in range(8):
                        nc.tensor.matmul(
                            pp[:], rx[:, k, :],
                            wpj[:, k, 512 * n2:512 * (n2 + 1)],
                            start=(k == 0), stop=(k == 7))
                    ob = stg.tile([128, 512], f32, tag="ob", bufs=2,
                                  name=f"obC_{n2}")
                    nc.vector.tensor_copy(ob[:], pp[:])
                    nc.sync.dma_start(
                        out_d[384:512, 512 * n2:512 * (n2 + 1)], ob[:])

    if split_waits:
        _split_multi_waits(nc)
    return nc


def make_aux_inputs():
    ident = np.eye(128, dtype=BF16)
    k_idx = np.arange(128)[:, None]
    q_idx = np.arange(QB)[None, :]
    masks = np.stack(
        [((k_idx + 128 * d) <= q_idx).astype(BF16) for d in range(4)], axis=0)
    return ident, masks


def make_in_maps(x, w_qkv, w_proj, seq=SEQ):
    x = np.asarray(x, dtype=np.float32).reshape(seq, D_MODEL)
    w_qkv = np.asarray(w_qkv, dtype=np.float32)
    w_proj = np.asarray(w_proj, dtype=np.float32)
    ident, masks = make_aux_inputs()
    in_maps = []
    for i in range(N_CORES):
        sl = slice(CPC * i, CPC * (i + 1))
        w_slice = np.concatenate(
            [w_qkv[:, sl], w_qkv[:, D_MODEL:][:, sl],
             w_qkv[:, 2 * D_MODEL:][:, sl]], axis=1)
        in_maps.append({
            "x": x,
            "w_slice": np.ascontiguousarray(w_slice),
            "w_proj": w_proj,
            "ident": ident,
            "masks": masks,
        })
    return in_maps


_NC_CACHE = {}


def kernel(x, w_qkv, w_proj):
    """Full inputs in, full output out. Shards internally across 8 cores."""
    try:
        import os
        import jax
        jax.config.update("jax_compilation_cache_dir",
                          os.path.expanduser("~/.cache/jax_bass_kernel"))
        jax.config.update("jax_persistent_cache_min_compile_time_secs", 0.0)
    except Exception:
        pass
    from concourse.bass_utils import run_bass_kernel_spmd

    x = np.asarray(x, dtype=np.float32)
    batch = x.shape[0]
    seq = x.shape[1]
    if seq not in _NC_CACHE:
        _NC_CACHE[seq] = build_nc(seq)
    nc = _NC_CACHE[seq]
    in_maps = make_in_maps(x, w_qkv, w_proj, seq=seq)
    res = run_bass_kernel_spmd(nc, in_maps, list(range(N_CORES)))
    # core j's rows are interleaved: local row 64*n + i (n = q-block) is
    # global row 512*n + 64*j + i
    out = np.empty((seq, D_MODEL), dtype=np.float32)
    outr = out.reshape(seq // QB, N_CORES, 64, D_MODEL)
    for j in range(N_CORES):
        outr[:, j] = res.results[j]["out"].reshape(seq // QB, 64, D_MODEL)
    return out.reshape(batch, seq, D_MODEL).astype(np.float32)



# revision 64
# speedup vs baseline: 1.0061x; 1.0061x over previous
"""Causal self-attention (d_model=1024, n_head=16, seq=4096) on 8 trn2 cores.

Sharding: tensor-parallel over heads (2 heads/core) for QKV + attention.
The output re-shard uses EIGHT small AllToAlls (one per 512-row q-block)
over an INTERLEAVED row partition -- core j owns rows {512*n + 64*j + i} --
so each collective fires right after its q-block's softmax-normalize and
overlaps the remaining attention.  The received rows' output projection is
braided into late attention blocks as PE filler; only the last q-block's
collective + projection remain in the serial tail.  The host reorders the
interleaved row shards.

Per-core layout (bf16 into the PE, fp32 PSUM accumulation):
  - x^T built via PE identity-matmul transposes (the d_model contraction
    needs x in [c, T] layout for both qkv operands).
  - qkv^T = w_slice.T @ x^T lands directly in [chan, T] layout, so qT/kT
    are exactly the lhsT/rhs of the score matmul (scores^T = K Q^T), and
    V' (normal orientation + a ones column per head) comes from small PE
    transposes.
  - softmax without max-subtraction (scores ~ N(0,1): exp cannot overflow
    fp32).  The PV matmul runs in [q, d] orientation (lhsT = P^T tile,
    rhs = V'), so each accumulation step moves only N=65 output columns
    (matmul cost is out-free-size cycles) -- half the PE cycles of the
    [d, q] orientation -- and the softmax denominator lands as a
    per-partition scalar, normalized with reciprocal + tensor_scalar
    multiplies (no cross-partition broadcast needed).  One PSUM
    accumulation group per bank: start marks the whole 2KB zero-region
    pending-zero; the four 128-row q-subtiles share it legally.
  - receivers transpose the [q, c] rows back to the projection's lhsT
    layout with small PE identity matmuls.
  - causal masking: only lower-triangle k-tiles are computed; diagonal
    tiles are masked by a precomputed 0/1 multiply after the exp.
  - emission is braided: prep for block n+1 (x load/transpose/qkv/V') and
    late projection pieces are interleaved INSIDE the attention groups of
    q-block n (between the score matmuls and the exp/mask/PV chain), so
    the PE stream has work while ACT runs the exps.  PSUM->SBUF copies run
    on DVE; x bf16 casts on gpsimd; exp is the only ACT-engine work.
  - collective n is issued one block late (after block n+1's x casts) so
    its wait never head-of-line blocks the Pool queue.
"""

import sys
import types

import numpy as np
import ml_dtypes

D_MODEL = 1024
N_HEAD = 16
SEQ = 4096
N_CORES = 8
D_HEAD = 64
CPC = 128            # channels per core (2 heads x 64)
QB = 512             # attention q-block width
BF16 = ml_dtypes.bfloat16
XBAR_FROM_BLOCK = 99   # blocks >= this: x^T via ACT-queue xbar into scratch
PBC_NORM = False       # normalize broadcast via gpsimd partition_broadcast


def _install_compat_patches():
    """Stub antenv.axon_hooks (absent in this container) so
    run_bass_kernel_spmd's trace path degrades instead of ImportError."""
    if "antenv.axon_hooks" not in sys.modules:
        mod = types.ModuleType("antenv.axon_hooks")
        mod.get_axon_ntff_profile_hook = lambda: None
        sys.modules["antenv.axon_hooks"] = mod


def _split_multi_waits(nc):
    """The nix walrus here accepts at most ONE sync-wait per instruction
    (setupSyncWait: 'Too many sync wait commands').  Hoist extra waits onto
    same-engine NoOps inserted immediately before the instruction — engine
    streams execute in program order, so semantics are unchanged."""
    import concourse.mybir as mybir

    n = 0
    for fn in nc.m.functions:
        for bb in fn.blocks:
            insts = bb.instructions
            out = []
            for inst in insts:
                si = getattr(inst, "sync_info", None)
                waits = list(si.on_wait) if si is not None else []
                if len(waits) > 1:
                    si.on_wait.clear()
                    for w in waits[:-1]:
                        n += 1
                        nop = mybir.InstNoOp(name=f"I-WSPLIT{n}", ins=[], outs=[])
                        nop.engine = inst.engine
                        nop.sync_info = mybir.SyncInfo(on_wait=[w], on_update=[])
                        out.append(nop)
                    si.on_wait.append(waits[-1])
                out.append(inst)
            bb.instructions = out


def build_nc(seq=SEQ, use_collective=True, split_waits=True):
    """Build the single-core SPMD program (identical on all 8 cores)."""
    import concourse.bass as bass
    import concourse.mybir as mybir
    from concourse.tile import TileContext

    _install_compat_patches()

    f32 = mybir.dt.float32
    bf16 = mybir.dt.bfloat16
    AFT = mybir.ActivationFunctionType

    from concourse import library_config

    nT = seq // 128       # T-tiles
    nQB = seq // QB       # attention q-blocks
    SW = seq // N_CORES   # AllToAll shard width (output rows per core)

    nc = bass.Bass("TRN2", target_bir_lowering=False, debug=False,
                   num_devices=N_CORES)
    x_d = nc.dram_tensor("x", [seq, D_MODEL], f32, kind="ExternalInput").ap()
    wq_d = nc.dram_tensor("w_slice", [D_MODEL, 3 * CPC], f32,
                          kind="ExternalInput").ap()
    wp_d = nc.dram_tensor("w_proj", [D_MODEL, D_MODEL], bf16,
                          kind="ExternalInput").ap()
    id_d = nc.dram_tensor("ident", [128, 128], bf16, kind="ExternalInput").ap()
    mk_d = nc.dram_tensor("masks", [4, 128, QB], bf16,
                          kind="ExternalInput").ap()
    out_d = nc.dram_tensor("out", [SW, D_MODEL], f32,
                           kind="ExternalOutput").ap()

    with TileContext(nc) as tc:
        with (
            tc.tile_pool(name="per", bufs=1) as per,
            tc.tile_pool(name="stg", bufs=2) as stg,
            tc.tile_pool(name="dram", bufs=1, space="DRAM") as dram,
        ):
            qT = per.tile([128, seq], bf16)      # [2 heads x 64 d, T]
            kT = per.tile([128, seq], bf16)
            Vp = per.tile([128, nT, 130], bf16)  # V' tiles: [v_h0|1|v_h1|1]
            wqkv = per.tile([128, 8, 3 * CPC], bf16)
            wpj = per.tile([128, 8, D_MODEL], bf16)
            iden = per.tile([128, 128], bf16)
            mks = per.tile([128, 4, QB], bf16)
            ones = per.tile([128, 64], f32)

            nc.scalar.dma_start(iden[:], id_d[:])
            nc.any.memset(ones[:], 1.0)
            nc.any.memset(Vp[:, :, 64:65], 1.0)
            nc.any.memset(Vp[:, :, 129:130], 1.0)

            # (weight staging happens inside the xstg pool below)

            # per-q-block AllToAll buffers over the INTERLEAVED row shard:
            # core j owns rows {512*n + 64*j + i}; chunk j of a2a_in[n] is
            # this core's 2 heads of that 64-row slice, so collective n can
            # fire right after q-block n's normalize.
            a2a_in = [dram.tile([4, 2, 64, 128], bf16, name=f"a2ain_{n}")
                      for n in range(nQB)]
            a2a_out = [dram.tile([8, 64, 128], bf16, name=f"a2aout_{n}")
                       for n in range(nQB)]

            # ---- phases 0-2, braided emission ------------------------
            # Engines execute their scheduled streams in static order, so
            # overlap must be built into emission order: the prep work
            # (x-load/transpose/qkv/V') for block n+1 is interleaved chunk-
            # by-chunk between the attention groups of q-block n.  Attention
            # qb=n depends only on qkv blocks 0..n, so each braid is legal.
            # PSUM banks: pA 2x1 + sT 2x2 + yt0 1 + yt1 1 = 8
            with (
                tc.tile_pool(name="xp", bufs=1) as xp,
                tc.tile_pool(name="xstg", bufs=3) as xstg,
                tc.tile_pool(name="ps", bufs=2, space="PSUM") as ps,
            ):
                xT = xp.tile([128, 8, seq], bf16)   # [c-chunk part, chunk, T]

                def wqkv_stage():
                    for k in range(8):
                        wtmp = xstg.tile([128, 3 * CPC], f32, tag="wq",
                                         bufs=4, name=f"wtmp_{k}")
                        nc.sync.dma_start(wtmp[:],
                                          wq_d[128 * k:128 * (k + 1), :])
                        nc.vector.tensor_copy(wqkv[:, k, :], wtmp[:])

                def prep_chunks(n):
                    """Emit-closures for block n: loads, x^T xbar, qkv^T, V'."""
                    state = {}

                    def loads():
                        xbs = []
                        for u in range(4):
                            t = 4 * n + u
                            xf = xstg.tile([128, D_MODEL], f32, tag="xf",
                                           bufs=8, name=f"xf_{t}")
                            nc.sync.dma_start(xf[:],
                                              x_d[128 * t:128 * (t + 1), :])
                            xb = xstg.tile([128, D_MODEL], bf16, tag="xb",
                                           bufs=8, name=f"xb_{t}")
                            nc.gpsimd.tensor_copy(xb[:], xf[:])
                            xbs.append(xb)
                        state["xbs"] = xbs

                    def trans(j):
                        # j indexes (x-tile u = j//2, c-chunk quad a = j%2):
                        # one PSUM tile holds 4 c-chunk transposes of a
                        # single x-tile, so work starts after its one load
                        def emit():
                            u, a = divmod(j, 2)
                            tp = ps.tile([128, 512], f32, tag="pA",
                                         name=f"tp_{n}_{j}")
                            for c in range(4):
                                nc.tensor.matmul(
                                    tp[:, 128 * c:128 * (c + 1)],
                                    state["xbs"][u][:, 128 * (4 * a + c):
                                                    128 * (4 * a + c + 1)],
                                    iden[:], start=True, stop=True)
                            nc.vector.tensor_copy(
                                xT[:, 4 * a:4 * (a + 1),
                                   128 * (4 * n + u):
                                   128 * (4 * n + u + 1)],
                                tp[:])
                        return emit

                    def qkv(m):
                        def emit():
                            qp = ps.tile([128, 512], f32, tag="pA",
                                         name=f"qp_{n}_{m}")
                            for k in range(8):
                                nc.tensor.matmul(
                                    qp[:],
                                    wqkv[:, k, 128 * m:128 * (m + 1)],
                                    xT[:, k, 512 * n:512 * (n + 1)],
                                    start=(k == 0), stop=(k == 7))
                            if m == 0:
                                nc.vector.tensor_copy(
                                    qT[:, 512 * n:512 * (n + 1)], qp[:])
                            elif m == 1:
                                nc.vector.tensor_copy(
                                    kT[:, 512 * n:512 * (n + 1)], qp[:])
                            else:
                                vs = xstg.tile([128, 512], bf16, tag="vs",
                                               bufs=2, name=f"vs_{n}")
                                nc.vector.tensor_copy(vs[:], qp[:])
                                state["vs"] = vs
                        return emit

                    def vtr(u):
                        def emit():
                            t = 4 * n + u
                            vs = state["vs"]
                            # separate PSUM tiles per head: PE-write plus
                            # DVE-read of one PSUM bank is a HW fault
                            vp0 = ps.tile([128, 64], f32, tag="pA",
                                          name=f"vp0_{t}")
                            vp1 = ps.tile([128, 64], f32, tag="pA",
                                          name=f"vp1_{t}")
                            nc.tensor.matmul(
                                vp0[:], vs[0:64, 128 * u:128 * (u + 1)],
                                iden[0:64, 0:64], start=True, stop=True)
                            nc.tensor.matmul(
                                vp1[:], vs[64:128, 128 * u:128 * (u + 1)],
                                iden[64:128, 64:128], start=True, stop=True)
                            nc.vector.tensor_copy(Vp[:, t, 0:64], vp0[:])
                            nc.vector.tensor_copy(Vp[:, t, 65:129], vp1[:])
                        return emit

                    return ([loads] + [trans(j) for j in range(8)]
                            + [qkv(m) for m in range(3)]
                            + [vtr(u) for u in range(4)])

                carry = {}

                def off_of(qb, kt):
                    d = kt - 4 * qb
                    return 128 * d if d >= 0 else 0

                def emit_scores(qb, g):
                    # h-inner MM order: consecutive score matmuls use
                    # disjoint PE row-groups (h0 rows 0-63, h1 rows 64-127)
                    # so the 16x32x32-subarray PE overlaps them
                    sps = [ps.tile([128, 2 * QB], f32, tag="sT",
                                   name=f"sp_{qb}_{g}_{h}")
                           for h in (0, 1)]
                    for u in (0, 1):
                        kt = 2 * g + u
                        o = off_of(qb, kt)
                        for h in (0, 1):
                            nc.tensor.matmul(
                                sps[h][:, QB * u + o:QB * (u + 1)],
                                kT[64 * h:64 * (h + 1),
                                   128 * kt:128 * (kt + 1)],
                                qT[64 * h:64 * (h + 1),
                                   QB * qb + o:QB * (qb + 1)],
                                start=True, stop=True)
                    return sps

                def hoist_scores(qb):
                    # pre-emit the NEXT block's first score matmuls inside
                    # the current block's last-group filler, so the exp
                    # stream crosses the block boundary without a bubble
                    def emit():
                        carry[qb] = emit_scores(qb, 0)
                    return emit

                def attention_groups(qb, ytps, fill=None):
                    nkt = 4 * (qb + 1)

                    def group(g):
                        # diagonal k-tiles (d = kt-4qb >= 0) only attend to
                        # q >= 128d: trim score MM / exp / mask / yT MM to
                        # the valid column range [128d, QB).  q-cols below
                        # that are fully masked and, because kt=0 always
                        # covers the full width with start=True, never read.
                        def off(kt):
                            return off_of(qb, kt)

                        def emit():
                            if g == 0 and qb in carry:
                                sps = carry.pop(qb)
                            else:
                                sps = emit_scores(qb, g)
                            if fill is not None:
                                fill(g)
                            diag = off(2 * g) > 0 or off(2 * g + 1) > 0
                            for h in (0, 1):
                                pt = stg.tile([128, 2 * QB], bf16, tag="pT",
                                              bufs=3, name=f"pt_{qb}_{g}_{h}")
                                if diag:
                                    for u in (0, 1):
                                        o = off(2 * g + u)
                                        nc.scalar.activation(
                                            pt[:, QB * u + o:QB * (u + 1)],
                                            sps[h][:, QB * u + o:QB * (u + 1)],
                                            AFT.Exp, scale=0.125)
                                else:
                                    nc.scalar.activation(pt[:], sps[h][:],
                                                         AFT.Exp, scale=0.125)
                                for u in (0, 1):
                                    kt = 2 * g + u
                                    d = kt - 4 * qb
                                    o = off(kt)
                                    if d >= 0:
                                        w = min(o + 128, QB)
                                        nc.vector.tensor_mul(
                                            pt[:, QB * u + o:QB * u + w],
                                            pt[:, QB * u + o:QB * u + w],
                                            mks[:, d, o:w])
                                    # PV in [q, d] orientation: N=65 per
                                    # accumulation step instead of N=512
                                    # (matmul cost is out-free-size cycles).
                                    # One accumulation group per PSUM bank:
                                    # start marks the whole 2KB zero-region
                                    # pending-zero (first touch of each s
                                    # slot overwrites, later ones accumulate)
                                    for s in range(max(0, d), 4):
                                        nc.tensor.matmul(
                                            ytps[h][:, s, :],
                                            pt[:, QB * u + 128 * s:
                                               QB * u + 128 * (s + 1)],
                                            Vp[:, kt,
                                               65 * h:65 * (h + 1)],
                                            start=(kt == 0 and s == 0),
                                            stop=(kt == nkt - 1))
                        return emit

                    return [group(g) for g in range(nkt // 2)]

                def normalize(qb, ytps):
                    # y is [q, d] with q on partitions, so 1/denom is a
                    # per-partition scalar multiply -- no broadcast needed
                    ysn = stg.tile([128, 4, 128], bf16, tag="ysn", bufs=2,
                                   name=f"ysn_{qb}")
                    for h in (0, 1):
                        yq = stg.tile([128, 4, 66], f32, tag="dn", bufs=4,
                                      name=f"yq_{qb}_{h}")
                        nc.vector.tensor_copy(yq[:, :, 0:65], ytps[h][:])
                        nc.vector.reciprocal(yq[:, :, 65:66],
                                             yq[:, :, 64:65])
                        for s in range(4):
                            nc.vector.tensor_scalar_mul(
                                ysn[:, s, 64 * h:64 * (h + 1)],
                                yq[:, s, 0:64], yq[:, s, 65:66])
                    # stage this q-block's interleaved AllToAll rows:
                    # chunk j = (qsub j//2, partitions 64*(j%2)..) holds
                    # global rows {512*qb + 64*j + i} as [64 q, 128 c]
                    for jj in (0, 1):
                        nc.sync.dma_start(
                            a2a_in[qb][:, jj].rearrange("s p c -> p s c"),
                            ysn[64 * jj:64 * (jj + 1), :, :])

                def wpj_chunk(k):
                    def emit():
                        # w_proj ships host-cast to bf16: direct DMA into
                        # the weight tile, no staging or cast
                        nc.sync.dma_start(wpj[:, k, :],
                                          wp_d[128 * k:128 * (k + 1), :])
                    return emit

                def collective(qb):
                    def emit():
                        if use_collective:
                            nc.gpsimd.collective_compute(
                                "AllToAll", mybir.AluOpType.bypass,
                                ins=[a2a_in[qb].opt()],
                                outs=[a2a_out[qb].opt()],
                                replica_groups=[list(range(N_CORES))])
                        else:
                            # timing-model stand-in (TimelineSim can't
                            # execute collectives): DRAM->DRAM copy
                            nc.sync.dma_start(a2a_out[qb].opt(),
                                              a2a_in[qb].opt())
                    return emit

                def rx_piece(p):
                    """Projection for the 128 interleaved rows of q-blocks
                    2p, 2p+1 (this core's shard of those blocks); braided
                    into a late attention block whose collectives are done."""
                    st = {}

                    def rx_loads():
                        rxq = stg.tile([64, 2, 8, 128], bf16, tag="rxq",
                                       bufs=2, name=f"rxq_{p}")
                        for e in (0, 1):
                            nc.sync.dma_start(
                                rxq[:, e],
                                a2a_out[2 * p + e].rearrange("i p c -> p i c"))
                        st["rxq"] = rxq

                    def rtr():
                        # received rows are [q, c]; transpose per 128-c chunk
                        # into the projection's lhsT layout [c, q]
                        rxT = stg.tile([128, 8, 128], bf16, tag="rx",
                                       bufs=2, name=f"rxT_{p}")
                        for e in (0, 1):
                            ytp = ps.tile([128, 8, 64], f32, tag="pA",
                                          name=f"ytp_{p}_{e}")
                            for i in range(8):
                                nc.tensor.matmul(
                                    ytp[:, i, :], st["rxq"][:, e, i, :],
                                    iden[0:64, 0:64], start=True, stop=True)
                            nc.vector.tensor_copy(
                                rxT[:, :, 64 * e:64 * (e + 1)], ytp[:])
                        st["rx"] = rxT

                    def half(n2):
                        def emit():
                            pp = ps.tile([128, 512], f32, tag="pA",
                                         name=f"pp_{p}_{n2}")
                            for k in range(8):
                                nc.tensor.matmul(
                                    pp[:],
                                    st["rx"][:, k, :],
                                    wpj[:, k, 512 * n2:512 * (n2 + 1)],
                                    start=(k == 0), stop=(k == 7))
                            ob = stg.tile([128, 512], f32, tag="ob", bufs=2,
                                          name=f"ob_{p}_{n2}")
                            nc.vector.tensor_copy(ob[:], pp[:])
                            nc.sync.dma_start(
                                out_d[128 * p:128 * (p + 1),
                                      512 * n2:512 * (n2 + 1)], ob[:])
                        return emit

                    return [rx_loads, rtr, half(0), half(1)]

                p0 = prep_chunks(0)
                p0[0]()           # stage-0 x loads lead the DMA queues
                wqkv_stage()
                for m in range(4):
                    # masks load after the startup-critical x/w DMAs
                    nc.scalar.dma_start(mks[:, m, :], mk_d[m])
                for c in p0[1:]:
                    c()
                for n in range(nQB):
                    ytps = [ps.tile([128, 4, 65], f32, tag=f"yt{h}", bufs=1,
                                    name=f"yt{h}_{n}") for h in (0, 1)]
                    pend = prep_chunks(n + 1) if n + 1 < nQB else []
                    if n >= 1:
                        # issue q-block n-1's AllToAll one block late so its
                        # wait (on the staging DMAs) is already satisfied and
                        # never head-of-line blocks the Pool queue
                        pend = pend[:1] + [collective(n - 1)] + pend[1:]
                    if 3 <= n <= 4:
                        # stage w_proj in blocks 3-4: late enough to stay
                        # clear of the prep-limited early blocks, done
                        # before piece 0 consumes it at block 5
                        ks = {3: (0, 1, 2, 3), 4: (4, 5, 6, 7)}[n]
                        pend = pend + [wpj_chunk(k) for k in ks]
                    if n >= 5:
                        # piece p covers q-blocks 2p, 2p+1; its collectives
                        # completed >= 2 blocks ago
                        pend = pend + rx_piece(n - 5)
                    if n + 1 < nQB:
                        pend = pend + [hoist_scores(n + 1)]
                    st = {"ci": 0}

                    def fill(gi):
                        want = (gi + 1) * len(pend) // (2 * (n + 1))
                        while st["ci"] < want:
                            pend[st["ci"]]()
                            st["ci"] += 1

                    groups = attention_groups(n, ytps, fill)
                    for g in groups:
                        g()
                    while st["ci"] < len(pend):
                        pend[st["ci"]]()
                        st["ci"] += 1
                    normalize(n, ytps)
                collective(nQB - 1)()

            # ---- tail: projection piece for q-blocks 6,7 ------------------
            with tc.tile_pool(name="psC", bufs=1, space="PSUM") as psC:
                rxq = stg.tile([64, 2, 8, 128], bf16, tag="rxq", bufs=2,
                               name="rxq_3")
                for e in (0, 1):
                    nc.sync.dma_start(
                        rxq[:, e],
                        a2a_out[6 + e].rearrange("i p c -> p i c"))
                rx = stg.tile([128, 8, 128], bf16, tag="rx", bufs=2,
                              name="rxT_3")
                for e in (0, 1):
                    ytp = psC.tile([128, 8, 64], f32, tag="ytpC",
                                   name=f"ytpC_{e}")
                    for i in range(8):
                        nc.tensor.matmul(
                            ytp[:, i, :], rxq[:, e, i, :],
                            iden[0:64, 0:64], start=True, stop=True)
                    nc.vector.tensor_copy(
                        rx[:, :, 64 * e:64 * (e + 1)], ytp[:])
                for n2 in (0, 1):
                    pp = psC.tile([128, 512], f32, tag="ppC",
                                  name=f"ppC_{n2}")
                    for k in range(8):
                        nc.tensor.matmul(
                            pp[:], rx[:, k, :],
                            wpj[:, k, 512 * n2:512 * (n2 + 1)],
                            start=(k == 0), stop=(k == 7))
                    ob = stg.tile([128, 512], f32, tag="ob", bufs=2,
                                  name=f"obC_{n2}")
                    nc.vector.tensor_copy(ob[:], pp[:])
                    nc.sync.dma_start(
                        out_d[384:512, 512 * n2:512 * (n2 + 1)], ob[:])

    if split_waits:
        _split_multi_waits(nc)
    return nc


def make_aux_inputs():
    ident = np.eye(128, dtype=BF16)
    k_idx = np.arange(128)[:, None]
    q_idx = np.arange(QB)[None, :]
    masks = np.stack(
        [((k_idx + 128 * d) <= q_idx).astype(BF16) for d in range(4)], axis=0)
    return ident, masks


def make_in_maps(x, w_qkv, w_proj, seq=SEQ):
    x = np.asarray(x, dtype=np.float32).reshape(seq, D_MODEL)
    w_qkv = np.asarray(w_qkv, dtype=np.float32)
    w_proj = np.asarray(w_proj, dtype=np.float32).astype(BF16)
    ident, masks = make_aux_inputs()
    in_maps = []
    for i in range(N_CORES):
        sl = slice(CPC * i, CPC * (i + 1))
        w_slice = np.concatenate(
            [w_qkv[:, sl], w_qkv[:, D_MODEL:][:, sl],
             w_qkv[:, 2 * D_MODEL:][:, sl]], axis=1)
        in_maps.append({
            "x": x,
            "w_slice": np.ascontiguousarray(w_slice),
            "w_proj": w_proj,
            "ident": ident,
            "masks": masks,
        })
    return in_maps


_NC_CACHE = {}


def kernel(x, w_qkv, w_proj):
    """Full inputs in, full output out. Shards internally across 8 cores."""
    try:
        import os
        import jax
        jax.config.update("jax_compilation_cache_dir",
                          os.path.expanduser("~/.cache/jax_bass_kernel"))
        jax.config.update("jax_persistent_cache_min_compile_time_secs", 0.0)
    except Exception:
        pass
    from concourse.bass_utils import run_bass_kernel_spmd

    x = np.asarray(x, dtype=np.float32)
    batch = x.shape[0]
    seq = x.shape[1]
    if seq not in _NC_CACHE:
        _NC_CACHE[seq] = build_nc(seq)
    nc = _NC_CACHE[seq]
    in_maps = make_in_maps(x, w_qkv, w_proj, seq=seq)
    res = run_bass_kernel_spmd(nc, in_maps, list(range(N_CORES)))
    # core j's rows are interleaved: local row 64*n + i (n = q-block) is
    # global row 512*n + 64*j + i
    out = np.empty((seq, D_MODEL), dtype=np.float32)
    outr = out.reshape(seq // QB, N_CORES, 64, D_MODEL)
    for j in range(N_CORES):
        outr[:, j] = res.results[j]["out"].reshape(seq // QB, 64, D_MODEL)
    return out.reshape(batch, seq, D_MODEL).astype(np.float32)



# revision 65
# speedup vs baseline: 1.0256x; 1.0194x over previous
"""Causal self-attention (d_model=1024, n_head=16, seq=4096) on 8 trn2 cores.

Sharding: tensor-parallel over heads (2 heads/core) for QKV + attention.
The output re-shard uses EIGHT small AllToAlls (one per 512-row q-block)
over an INTERLEAVED row partition -- core j owns rows {512*n + 64*j + i} --
so each collective fires right after its q-block's softmax-normalize and
overlaps the remaining attention.  The received rows' output projection is
braided into late attention blocks as PE filler; only the last q-block's
collective + projection remain in the serial tail.  The host reorders the
interleaved row shards.

Per-core layout (bf16 into the PE, fp32 PSUM accumulation):
  - x^T built via PE identity-matmul transposes (the d_model contraction
    needs x in [c, T] layout for both qkv operands).
  - qkv^T = w_slice.T @ x^T lands directly in [chan, T] layout, so qT/kT
    are exactly the lhsT/rhs of the score matmul (scores^T = K Q^T), and
    V' (normal orientation + a ones column per head) comes from small PE
    transposes.
  - softmax without max-subtraction (scores ~ N(0,1): exp cannot overflow
    fp32).  The PV matmul runs in [q, d] orientation (lhsT = P^T tile,
    rhs = V'), so each accumulation step moves only N=65 output columns
    (matmul cost is out-free-size cycles) -- half the PE cycles of the
    [d, q] orientation -- and the softmax denominator lands as a
    per-partition scalar, normalized with reciprocal + tensor_scalar
    multiplies (no cross-partition broadcast needed).  One PSUM
    accumulation group per bank: start marks the whole 2KB zero-region
    pending-zero; the four 128-row q-subtiles share it legally.
  - receivers transpose the [q, c] rows back to the projection's lhsT
    layout with small PE identity matmuls.
  - causal masking: only lower-triangle k-tiles are computed; diagonal
    tiles are masked by a precomputed 0/1 multiply after the exp.
  - emission is braided: prep for block n+1 (x load/transpose/qkv/V') and
    late projection pieces are interleaved INSIDE the attention groups of
    q-block n (between the score matmuls and the exp/mask/PV chain), so
    the PE stream has work while ACT runs the exps.  PSUM->SBUF copies run
    on DVE; x bf16 casts on gpsimd; exp is the only ACT-engine work.
  - collective n is issued one block late (after block n+1's x casts) so
    its wait never head-of-line blocks the Pool queue.
"""

import sys
import types

import numpy as np
import ml_dtypes

D_MODEL = 1024
N_HEAD = 16
SEQ = 4096
N_CORES = 8
D_HEAD = 64
CPC = 128            # channels per core (2 heads x 64)
QB = 512             # attention q-block width
BF16 = ml_dtypes.bfloat16
XBAR_FROM_BLOCK = 99   # blocks >= this: x^T via ACT-queue xbar into scratch
PBC_NORM = False       # normalize broadcast via gpsimd partition_broadcast


def _install_compat_patches():
    """Stub antenv.axon_hooks (absent in this container) so
    run_bass_kernel_spmd's trace path degrades instead of ImportError."""
    if "antenv.axon_hooks" not in sys.modules:
        mod = types.ModuleType("antenv.axon_hooks")
        mod.get_axon_ntff_profile_hook = lambda: None
        sys.modules["antenv.axon_hooks"] = mod


def _split_multi_waits(nc):
    """The nix walrus here accepts at most ONE sync-wait per instruction
    (setupSyncWait: 'Too many sync wait commands').  Hoist extra waits onto
    same-engine NoOps inserted immediately before the instruction — engine
    streams execute in program order, so semantics are unchanged."""
    import concourse.mybir as mybir

    n = 0
    for fn in nc.m.functions:
        for bb in fn.blocks:
            insts = bb.instructions
            out = []
            for inst in insts:
                si = getattr(inst, "sync_info", None)
                waits = list(si.on_wait) if si is not None else []
                if len(waits) > 1:
                    si.on_wait.clear()
                    for w in waits[:-1]:
                        n += 1
                        nop = mybir.InstNoOp(name=f"I-WSPLIT{n}", ins=[], outs=[])
                        nop.engine = inst.engine
                        nop.sync_info = mybir.SyncInfo(on_wait=[w], on_update=[])
                        out.append(nop)
                    si.on_wait.append(waits[-1])
                out.append(inst)
            bb.instructions = out


def build_nc(seq=SEQ, use_collective=True, split_waits=True):
    """Build the single-core SPMD program (identical on all 8 cores)."""
    import concourse.bass as bass
    import concourse.mybir as mybir
    from concourse.tile import TileContext

    _install_compat_patches()

    f32 = mybir.dt.float32
    bf16 = mybir.dt.bfloat16
    AFT = mybir.ActivationFunctionType

    from concourse import library_config

    nT = seq // 128       # T-tiles
    nQB = seq // QB       # attention q-blocks
    SW = seq // N_CORES   # AllToAll shard width (output rows per core)

    nc = bass.Bass("TRN2", target_bir_lowering=False, debug=False,
                   num_devices=N_CORES)
    x_d = nc.dram_tensor("x", [seq, D_MODEL], bf16, kind="ExternalInput").ap()
    wq_d = nc.dram_tensor("w_slice", [D_MODEL, 3 * CPC], f32,
                          kind="ExternalInput").ap()
    wp_d = nc.dram_tensor("w_proj", [D_MODEL, D_MODEL], bf16,
                          kind="ExternalInput").ap()
    id_d = nc.dram_tensor("ident", [128, 128], bf16, kind="ExternalInput").ap()
    mk_d = nc.dram_tensor("masks", [4, 128, QB], bf16,
                          kind="ExternalInput").ap()
    out_d = nc.dram_tensor("out", [SW, D_MODEL], f32,
                           kind="ExternalOutput").ap()

    with TileContext(nc) as tc:
        with (
            tc.tile_pool(name="per", bufs=1) as per,
            tc.tile_pool(name="stg", bufs=2) as stg,
            tc.tile_pool(name="dram", bufs=1, space="DRAM") as dram,
        ):
            qT = per.tile([128, seq], bf16)      # [2 heads x 64 d, T]
            kT = per.tile([128, seq], bf16)
            Vp = per.tile([128, nT, 130], bf16)  # V' tiles: [v_h0|1|v_h1|1]
            wqkv = per.tile([128, 8, 3 * CPC], bf16)
            wpj = per.tile([128, 8, D_MODEL], bf16)
            iden = per.tile([128, 128], bf16)
            mks = per.tile([128, 4, QB], bf16)
            ones = per.tile([128, 64], f32)

            nc.scalar.dma_start(iden[:], id_d[:])
            nc.any.memset(ones[:], 1.0)
            nc.any.memset(Vp[:, :, 64:65], 1.0)
            nc.any.memset(Vp[:, :, 129:130], 1.0)

            # (weight staging happens inside the xstg pool below)

            # per-q-block AllToAll buffers over the INTERLEAVED row shard:
            # core j owns rows {512*n + 64*j + i}; chunk j of a2a_in[n] is
            # this core's 2 heads of that 64-row slice, so collective n can
            # fire right after q-block n's normalize.
            a2a_in = [dram.tile([4, 2, 64, 128], bf16, name=f"a2ain_{n}")
                      for n in range(nQB)]
            a2a_out = [dram.tile([8, 64, 128], bf16, name=f"a2aout_{n}")
                       for n in range(nQB)]

            # ---- phases 0-2, braided emission ------------------------
            # Engines execute their scheduled streams in static order, so
            # overlap must be built into emission order: the prep work
            # (x-load/transpose/qkv/V') for block n+1 is interleaved chunk-
            # by-chunk between the attention groups of q-block n.  Attention
            # qb=n depends only on qkv blocks 0..n, so each braid is legal.
            # PSUM banks: pA 2x1 + sT 2x2 + yt0 1 + yt1 1 = 8
            with (
                tc.tile_pool(name="xp", bufs=1) as xp,
                tc.tile_pool(name="xstg", bufs=3) as xstg,
                tc.tile_pool(name="ps", bufs=2, space="PSUM") as ps,
            ):
                xT = xp.tile([128, 8, seq], bf16)   # [c-chunk part, chunk, T]

                def wqkv_stage():
                    for k in range(8):
                        wtmp = xstg.tile([128, 3 * CPC], f32, tag="wq",
                                         bufs=4, name=f"wtmp_{k}")
                        nc.sync.dma_start(wtmp[:],
                                          wq_d[128 * k:128 * (k + 1), :])
                        nc.vector.tensor_copy(wqkv[:, k, :], wtmp[:])

                def prep_chunks(n):
                    """Emit-closures for block n: loads, x^T xbar, qkv^T, V'."""
                    state = {}

                    def loads():
                        # x ships host-cast to bf16: DMA straight into the
                        # transpose staging tiles, no on-chip cast
                        xbs = []
                        for u in range(4):
                            t = 4 * n + u
                            xb = xstg.tile([128, D_MODEL], bf16, tag="xb",
                                           bufs=8, name=f"xb_{t}")
                            nc.sync.dma_start(xb[:],
                                              x_d[128 * t:128 * (t + 1), :])
                            xbs.append(xb)
                        state["xbs"] = xbs

                    def trans(j):
                        # j indexes (x-tile u = j//2, c-chunk quad a = j%2):
                        # one PSUM tile holds 4 c-chunk transposes of a
                        # single x-tile, so work starts after its one load
                        def emit():
                            u, a = divmod(j, 2)
                            tp = ps.tile([128, 512], f32, tag="pA",
                                         name=f"tp_{n}_{j}")
                            for c in range(4):
                                nc.tensor.matmul(
                                    tp[:, 128 * c:128 * (c + 1)],
                                    state["xbs"][u][:, 128 * (4 * a + c):
                                                    128 * (4 * a + c + 1)],
                                    iden[:], start=True, stop=True)
                            nc.vector.tensor_copy(
                                xT[:, 4 * a:4 * (a + 1),
                                   128 * (4 * n + u):
                                   128 * (4 * n + u + 1)],
                                tp[:])
                        return emit

                    def qkv(m):
                        def emit():
                            qp = ps.tile([128, 512], f32, tag="pA",
                                         name=f"qp_{n}_{m}")
                            for k in range(8):
                                nc.tensor.matmul(
                                    qp[:],
                                    wqkv[:, k, 128 * m:128 * (m + 1)],
                                    xT[:, k, 512 * n:512 * (n + 1)],
                                    start=(k == 0), stop=(k == 7))
                            if m == 0:
                                nc.vector.tensor_copy(
                                    qT[:, 512 * n:512 * (n + 1)], qp[:])
                            elif m == 1:
                                nc.vector.tensor_copy(
                                    kT[:, 512 * n:512 * (n + 1)], qp[:])
                            else:
                                vs = xstg.tile([128, 512], bf16, tag="vs",
                                               bufs=2, name=f"vs_{n}")
                                nc.vector.tensor_copy(vs[:], qp[:])
                                state["vs"] = vs
                        return emit

                    def vtr(u):
                        def emit():
                            t = 4 * n + u
                            vs = state["vs"]
                            # separate PSUM tiles per head: PE-write plus
                            # DVE-read of one PSUM bank is a HW fault
                            vp0 = ps.tile([128, 64], f32, tag="pA",
                                          name=f"vp0_{t}")
                            vp1 = ps.tile([128, 64], f32, tag="pA",
                                          name=f"vp1_{t}")
                            nc.tensor.matmul(
                                vp0[:], vs[0:64, 128 * u:128 * (u + 1)],
                                iden[0:64, 0:64], start=True, stop=True)
                            nc.tensor.matmul(
                                vp1[:], vs[64:128, 128 * u:128 * (u + 1)],
                                iden[64:128, 64:128], start=True, stop=True)
                            nc.vector.tensor_copy(Vp[:, t, 0:64], vp0[:])
                            nc.vector.tensor_copy(Vp[:, t, 65:129], vp1[:])
                        return emit

                    return ([loads] + [trans(j) for j in range(8)]
                            + [qkv(m) for m in range(3)]
                            + [vtr(u) for u in range(4)])

                carry = {}

                def off_of(qb, kt):
                    d = kt - 4 * qb
                    return 128 * d if d >= 0 else 0

                def emit_scores(qb, g):
                    # h-inner MM order: consecutive score matmuls use
                    # disjoint PE row-groups (h0 rows 0-63, h1 rows 64-127)
                    # so the 16x32x32-subarray PE overlaps them
                    sps = [ps.tile([128, 2 * QB], f32, tag="sT",
                                   name=f"sp_{qb}_{g}_{h}")
                           for h in (0, 1)]
                    for u in (0, 1):
                        kt = 2 * g + u
                        o = off_of(qb, kt)
                        for h in (0, 1):
                            nc.tensor.matmul(
                                sps[h][:, QB * u + o:QB * (u + 1)],
                                kT[64 * h:64 * (h + 1),
                                   128 * kt:128 * (kt + 1)],
                                qT[64 * h:64 * (h + 1),
                                   QB * qb + o:QB * (qb + 1)],
                                start=True, stop=True)
                    return sps

                def hoist_scores(qb):
                    # pre-emit the NEXT block's first score matmuls inside
                    # the current block's last-group filler, so the exp
                    # stream crosses the block boundary without a bubble
                    def emit():
                        carry[qb] = emit_scores(qb, 0)
                    return emit

                def attention_groups(qb, ytps, fill=None):
                    nkt = 4 * (qb + 1)

                    def group(g):
                        # diagonal k-tiles (d = kt-4qb >= 0) only attend to
                        # q >= 128d: trim score MM / exp / mask / yT MM to
                        # the valid column range [128d, QB).  q-cols below
                        # that are fully masked and, because kt=0 always
                        # covers the full width with start=True, never read.
                        def off(kt):
                            return off_of(qb, kt)

                        def emit():
                            if g == 0 and qb in carry:
                                sps = carry.pop(qb)
                            else:
                                sps = emit_scores(qb, g)
                            if fill is not None:
                                fill(g)
                            diag = off(2 * g) > 0 or off(2 * g + 1) > 0
                            for h in (0, 1):
                                pt = stg.tile([128, 2 * QB], bf16, tag="pT",
                                              bufs=3, name=f"pt_{qb}_{g}_{h}")
                                if diag:
                                    for u in (0, 1):
                                        o = off(2 * g + u)
                                        nc.scalar.activation(
                                            pt[:, QB * u + o:QB * (u + 1)],
                                            sps[h][:, QB * u + o:QB * (u + 1)],
                                            AFT.Exp, scale=0.125)
                                else:
                                    nc.scalar.activation(pt[:], sps[h][:],
                                                         AFT.Exp, scale=0.125)
                                for u in (0, 1):
                                    kt = 2 * g + u
                                    d = kt - 4 * qb
                                    o = off(kt)
                                    if d >= 0:
                                        w = min(o + 128, QB)
                                        nc.vector.tensor_mul(
                                            pt[:, QB * u + o:QB * u + w],
                                            pt[:, QB * u + o:QB * u + w],
                                            mks[:, d, o:w])
                                    # PV in [q, d] orientation: N=65 per
                                    # accumulation step instead of N=512
                                    # (matmul cost is out-free-size cycles).
                                    # One accumulation group per PSUM bank:
                                    # start marks the whole 2KB zero-region
                                    # pending-zero (first touch of each s
                                    # slot overwrites, later ones accumulate)
                                    for s in range(max(0, d), 4):
                                        nc.tensor.matmul(
                                            ytps[h][:, s, :],
                                            pt[:, QB * u + 128 * s:
                                               QB * u + 128 * (s + 1)],
                                            Vp[:, kt,
                                               65 * h:65 * (h + 1)],
                                            start=(kt == 0 and s == 0),
                                            stop=(kt == nkt - 1))
                        return emit

                    return [group(g) for g in range(nkt // 2)]

                def normalize(qb, ytps):
                    # y is [q, d] with q on partitions, so 1/denom is a
                    # per-partition scalar multiply -- no broadcast needed
                    ysn = stg.tile([128, 4, 128], bf16, tag="ysn", bufs=2,
                                   name=f"ysn_{qb}")
                    for h in (0, 1):
                        yq = stg.tile([128, 4, 66], f32, tag="dn", bufs=4,
                                      name=f"yq_{qb}_{h}")
                        nc.vector.tensor_copy(yq[:, :, 0:65], ytps[h][:])
                        nc.vector.reciprocal(yq[:, :, 65:66],
                                             yq[:, :, 64:65])
                        for s in range(4):
                            nc.vector.tensor_scalar_mul(
                                ysn[:, s, 64 * h:64 * (h + 1)],
                                yq[:, s, 0:64], yq[:, s, 65:66])
                    # stage this q-block's interleaved AllToAll rows:
                    # chunk j = (qsub j//2, partitions 64*(j%2)..) holds
                    # global rows {512*qb + 64*j + i} as [64 q, 128 c]
                    for jj in (0, 1):
                        nc.sync.dma_start(
                            a2a_in[qb][:, jj].rearrange("s p c -> p s c"),
                            ysn[64 * jj:64 * (jj + 1), :, :])

                def wpj_chunk(k):
                    def emit():
                        # w_proj ships host-cast to bf16: direct DMA into
                        # the weight tile, no staging or cast
                        nc.sync.dma_start(wpj[:, k, :],
                                          wp_d[128 * k:128 * (k + 1), :])
                    return emit

                def collective(qb):
                    def emit():
                        if use_collective:
                            nc.gpsimd.collective_compute(
                                "AllToAll", mybir.AluOpType.bypass,
                                ins=[a2a_in[qb].opt()],
                                outs=[a2a_out[qb].opt()],
                                replica_groups=[list(range(N_CORES))])
                        else:
                            # timing-model stand-in (TimelineSim can't
                            # execute collectives): DRAM->DRAM copy
                            nc.sync.dma_start(a2a_out[qb].opt(),
                                              a2a_in[qb].opt())
                    return emit

                def rx_piece(p):
                    """Projection for the 128 interleaved rows of q-blocks
                    2p, 2p+1 (this core's shard of those blocks); braided
                    into a late attention block whose collectives are done."""
                    st = {}

                    def rx_loads():
                        rxq = stg.tile([64, 2, 8, 128], bf16, tag="rxq",
                                       bufs=2, name=f"rxq_{p}")
                        for e in (0, 1):
                            nc.sync.dma_start(
                                rxq[:, e],
                                a2a_out[2 * p + e].rearrange("i p c -> p i c"))
                        st["rxq"] = rxq

                    def rtr():
                        # received rows are [q, c]; transpose per 128-c chunk
                        # into the projection's lhsT layout [c, q]
                        rxT = stg.tile([128, 8, 128], bf16, tag="rx",
                                       bufs=2, name=f"rxT_{p}")
                        for e in (0, 1):
                            ytp = ps.tile([128, 8, 64], f32, tag="pA",
                                          name=f"ytp_{p}_{e}")
                            for i in range(8):
                                nc.tensor.matmul(
                                    ytp[:, i, :], st["rxq"][:, e, i, :],
                                    iden[0:64, 0:64], start=True, stop=True)
                            nc.vector.tensor_copy(
                                rxT[:, :, 64 * e:64 * (e + 1)], ytp[:])
                        st["rx"] = rxT

                    def half(n2):
                        def emit():
                            pp = ps.tile([128, 512], f32, tag="pA",
                                         name=f"pp_{p}_{n2}")
                            for k in range(8):
                                nc.tensor.matmul(
                                    pp[:],
                                    st["rx"][:, k, :],
                                    wpj[:, k, 512 * n2:512 * (n2 + 1)],
                                    start=(k == 0), stop=(k == 7))
                            ob = stg.tile([128, 512], f32, tag="ob", bufs=2,
                                          name=f"ob_{p}_{n2}")
                            nc.vector.tensor_copy(ob[:], pp[:])
                            nc.sync.dma_start(
                                out_d[128 * p:128 * (p + 1),
                                      512 * n2:512 * (n2 + 1)], ob[:])
                        return emit

                    return [rx_loads, rtr, half(0), half(1)]

                p0 = prep_chunks(0)
                p0[0]()           # stage-0 x loads lead the DMA queues
                wqkv_stage()
                for m in range(4):
                    # masks load after the startup-critical x/w DMAs
                    nc.scalar.dma_start(mks[:, m, :], mk_d[m])
                for c in p0[1:]:
                    c()
                for n in range(nQB):
                    ytps = [ps.tile([128, 4, 65], f32, tag=f"yt{h}", bufs=1,
                                    name=f"yt{h}_{n}") for h in (0, 1)]
                    pend = prep_chunks(n + 1) if n + 1 < nQB else []
                    if n >= 1:
                        # issue q-block n-1's AllToAll one block late so its
                        # wait (on the staging DMAs) is already satisfied and
                        # never head-of-line blocks the Pool queue
                        pend = pend[:1] + [collective(n - 1)] + pend[1:]
                    if 3 <= n <= 4:
                        # stage w_proj in blocks 3-4: late enough to stay
                        # clear of the prep-limited early blocks, done
                        # before piece 0 consumes it at block 5
                        ks = {3: (0, 1, 2, 3), 4: (4, 5, 6, 7)}[n]
                        pend = pend + [wpj_chunk(k) for k in ks]
                    if n >= 5:
                        # piece p covers q-blocks 2p, 2p+1; its collectives
                        # completed >= 2 blocks ago
                        pend = pend + rx_piece(n - 5)
                    if n + 1 < nQB:
                        pend = pend + [hoist_scores(n + 1)]
                    st = {"ci": 0}

                    def fill(gi):
                        want = (gi + 1) * len(pend) // (2 * (n + 1))
                        while st["ci"] < want:
                            pend[st["ci"]]()
                            st["ci"] += 1

                    groups = attention_groups(n, ytps, fill)
                    for g in groups:
                        g()
                    while st["ci"] < len(pend):
                        pend[st["ci"]]()
                        st["ci"] += 1
                    normalize(n, ytps)
                collective(nQB - 1)()

            # ---- tail: projection piece for q-blocks 6,7 ------------------
            with tc.tile_pool(name="psC", bufs=1, space="PSUM") as psC:
                rxq = stg.tile([64, 2, 8, 128], bf16, tag="rxq", bufs=2,
                               name="rxq_3")
                for e in (0, 1):
                    nc.sync.dma_start(
                        rxq[:, e],
                        a2a_out[6 + e].rearrange("i p c -> p i c"))
                rx = stg.tile([128, 8, 128], bf16, tag="rx", bufs=2,
                              name="rxT_3")
                for e in (0, 1):
                    ytp = psC.tile([128, 8, 64], f32, tag="ytpC",
                                   name=f"ytpC_{e}")
                    for i in range(8):
                        nc.tensor.matmul(
                            ytp[:, i, :], rxq[:, e, i, :],
                            iden[0:64, 0:64], start=True, stop=True)
                    nc.vector.tensor_copy(
                        rx[:, :, 64 * e:64 * (e + 1)], ytp[:])
                for n2 in (0, 1):
                    pp = psC.tile([128, 512], f32, tag="ppC",
                                  name=f"ppC_{n2}")
                    for k in range(8):
                        nc.tensor.matmul(
                            pp[:], rx[:, k, :],
                            wpj[:, k, 512 * n2:512 * (n2 + 1)],
                            start=(k == 0), stop=(k == 7))
                    ob = stg.tile([128, 512], f32, tag="ob", bufs=2,
                                  name=f"obC_{n2}")
                    nc.vector.tensor_copy(ob[:], pp[:])
                    nc.sync.dma_start(
                        out_d[384:512, 512 * n2:512 * (n2 + 1)], ob[:])

    if split_waits:
        _split_multi_waits(nc)
    return nc


def make_aux_inputs():
    ident = np.eye(128, dtype=BF16)
    k_idx = np.arange(128)[:, None]
    q_idx = np.arange(QB)[None, :]
    masks = np.stack(
        [((k_idx + 128 * d) <= q_idx).astype(BF16) for d in range(4)], axis=0)
    return ident, masks


def make_in_maps(x, w_qkv, w_proj, seq=SEQ):
    x = np.asarray(x, dtype=np.float32).reshape(seq, D_MODEL).astype(BF16)
    w_qkv = np.asarray(w_qkv, dtype=np.float32)
    w_proj = np.asarray(w_proj, dtype=np.float32).astype(BF16)
    ident, masks = make_aux_inputs()
    in_maps = []
    for i in range(N_CORES):
        sl = slice(CPC * i, CPC * (i + 1))
        w_slice = np.concatenate(
            [w_qkv[:, sl], w_qkv[:, D_MODEL:][:, sl],
             w_qkv[:, 2 * D_MODEL:][:, sl]], axis=1)
        in_maps.append({
            "x": x,
            "w_slice": np.ascontiguousarray(w_slice),
            "w_proj": w_proj,
            "ident": ident,
            "masks": masks,
        })
    return in_maps


_NC_CACHE = {}


def kernel(x, w_qkv, w_proj):
    """Full inputs in, full output out. Shards internally across 8 cores."""
    try:
        import os
        import jax
        jax.config.update("jax_compilation_cache_dir",
                          os.path.expanduser("~/.cache/jax_bass_kernel"))
        jax.config.update("jax_persistent_cache_min_compile_time_secs", 0.0)
    except Exception:
        pass
    from concourse.bass_utils import run_bass_kernel_spmd

    x = np.asarray(x, dtype=np.float32)
    batch = x.shape[0]
    seq = x.shape[1]
    if seq not in _NC_CACHE:
        _NC_CACHE[seq] = build_nc(seq)
    nc = _NC_CACHE[seq]
    in_maps = make_in_maps(x, w_qkv, w_proj, seq=seq)
    res = run_bass_kernel_spmd(nc, in_maps, list(range(N_CORES)))
    # core j's rows are interleaved: local row 64*n + i (n = q-block) is
    # global row 512*n + 64*j + i
    out = np.empty((seq, D_MODEL), dtype=np.float32)
    outr = out.reshape(seq // QB, N_CORES, 64, D_MODEL)
    for j in range(N_CORES):
        outr[:, j] = res.results[j]["out"].reshape(seq // QB, 64, D_MODEL)
    return out.reshape(batch, seq, D_MODEL).astype(np.float32)



# revision 70
# speedup vs baseline: 1.0308x; 1.0051x over previous
"""Causal self-attention (d_model=1024, n_head=16, seq=4096) on 8 trn2 cores.

Sharding: tensor-parallel over heads (2 heads/core) for QKV + attention.
The output re-shard uses EIGHT small AllToAlls (one per 512-row q-block)
over an INTERLEAVED row partition -- core j owns rows {512*n + 64*j + i} --
so each collective fires right after its q-block's softmax-normalize and
overlaps the remaining attention.  The received rows' output projection is
braided into late attention blocks as PE filler; only the last q-block's
collective + projection remain in the serial tail.  The host reorders the
interleaved row shards.

Per-core layout (bf16 into the PE, fp32 PSUM accumulation):
  - x^T built via PE identity-matmul transposes (the d_model contraction
    needs x in [c, T] layout for both qkv operands).
  - qkv^T = w_slice.T @ x^T lands directly in [chan, T] layout, so qT/kT
    are exactly the lhsT/rhs of the score matmul (scores^T = K Q^T), and
    V' (normal orientation + a ones column per head) comes from small PE
    transposes.
  - softmax without max-subtraction (scores ~ N(0,1): exp cannot overflow
    fp32).  The PV matmul runs in [q, d] orientation (lhsT = P^T tile,
    rhs = V'), so each accumulation step moves only N=65 output columns
    (matmul cost is out-free-size cycles) -- half the PE cycles of the
    [d, q] orientation -- and the softmax denominator lands as a
    per-partition scalar, normalized with reciprocal + tensor_scalar
    multiplies (no cross-partition broadcast needed).  One PSUM
    accumulation group per bank: start marks the whole 2KB zero-region
    pending-zero; the four 128-row q-subtiles share it legally.
  - receivers transpose the [q, c] rows back to the projection's lhsT
    layout with small PE identity matmuls.
  - causal masking: only lower-triangle k-tiles are computed; diagonal
    tiles are masked by a precomputed 0/1 multiply after the exp.
  - emission is braided: prep for block n+1 (x load/transpose/qkv/V') and
    late projection pieces are interleaved INSIDE the attention groups of
    q-block n (between the score matmuls and the exp/mask/PV chain), so
    the PE stream has work while ACT runs the exps.  PSUM->SBUF copies run
    on DVE; x bf16 casts on gpsimd; exp is the only ACT-engine work.
  - collective n is issued one block late (after block n+1's x casts) so
    its wait never head-of-line blocks the Pool queue.
"""

import sys
import types

import numpy as np
import ml_dtypes

D_MODEL = 1024
N_HEAD = 16
SEQ = 4096
N_CORES = 8
D_HEAD = 64
CPC = 128            # channels per core (2 heads x 64)
QB = 512             # attention q-block width
BF16 = ml_dtypes.bfloat16
XBAR_FROM_BLOCK = 99   # blocks >= this: x^T via ACT-queue xbar into scratch
PBC_NORM = False       # normalize broadcast via gpsimd partition_broadcast


def _install_compat_patches():
    """Stub antenv.axon_hooks (absent in this container) so
    run_bass_kernel_spmd's trace path degrades instead of ImportError."""
    if "antenv.axon_hooks" not in sys.modules:
        mod = types.ModuleType("antenv.axon_hooks")
        mod.get_axon_ntff_profile_hook = lambda: None
        sys.modules["antenv.axon_hooks"] = mod


def _split_multi_waits(nc):
    """The nix walrus here accepts at most ONE sync-wait per instruction
    (setupSyncWait: 'Too many sync wait commands').  Hoist extra waits onto
    same-engine NoOps inserted immediately before the instruction — engine
    streams execute in program order, so semantics are unchanged."""
    import concourse.mybir as mybir

    n = 0
    for fn in nc.m.functions:
        for bb in fn.blocks:
            insts = bb.instructions
            out = []
            for inst in insts:
                si = getattr(inst, "sync_info", None)
                waits = list(si.on_wait) if si is not None else []
                if len(waits) > 1:
                    si.on_wait.clear()
                    for w in waits[:-1]:
                        n += 1
                        nop = mybir.InstNoOp(name=f"I-WSPLIT{n}", ins=[], outs=[])
                        nop.engine = inst.engine
                        nop.sync_info = mybir.SyncInfo(on_wait=[w], on_update=[])
                        out.append(nop)
                    si.on_wait.append(waits[-1])
                out.append(inst)
            bb.instructions = out


def build_nc(seq=SEQ, use_collective=True, split_waits=True):
    """Build the single-core SPMD program (identical on all 8 cores)."""
    import concourse.bass as bass
    import concourse.mybir as mybir
    from concourse.tile import TileContext

    _install_compat_patches()

    f32 = mybir.dt.float32
    bf16 = mybir.dt.bfloat16
    AFT = mybir.ActivationFunctionType

    from concourse import library_config

    nT = seq // 128       # T-tiles
    nQB = seq // QB       # attention q-blocks
    SW = seq // N_CORES   # AllToAll shard width (output rows per core)

    nc = bass.Bass("TRN2", target_bir_lowering=False, debug=False,
                   num_devices=N_CORES)
    x_d = nc.dram_tensor("x", [seq, D_MODEL], bf16, kind="ExternalInput").ap()
    wq_d = nc.dram_tensor("w_slice", [D_MODEL, 3 * CPC], bf16,
                          kind="ExternalInput").ap()
    wp_d = nc.dram_tensor("w_proj", [D_MODEL, D_MODEL], bf16,
                          kind="ExternalInput").ap()
    id_d = nc.dram_tensor("ident", [128, 128], bf16, kind="ExternalInput").ap()
    mk_d = nc.dram_tensor("masks", [4, 128, QB], bf16,
                          kind="ExternalInput").ap()
    out_d = nc.dram_tensor("out", [SW, D_MODEL], f32,
                           kind="ExternalOutput").ap()

    with TileContext(nc) as tc:
        with (
            tc.tile_pool(name="per", bufs=1) as per,
            tc.tile_pool(name="stg", bufs=2) as stg,
            tc.tile_pool(name="dram", bufs=1, space="DRAM") as dram,
        ):
            qT = per.tile([128, seq], bf16)      # [2 heads x 64 d, T]
            kT = per.tile([128, seq], bf16)
            Vp = per.tile([128, nT, 130], bf16)  # V' tiles: [v_h0|1|v_h1|1]
            wqkv = per.tile([128, 8, 3 * CPC], bf16)
            wpj = per.tile([128, 8, D_MODEL], bf16)
            iden = per.tile([128, 128], bf16)
            mks = per.tile([128, 4, QB], bf16)
            ones = per.tile([128, 64], f32)

            nc.scalar.dma_start(iden[:], id_d[:])
            nc.any.memset(ones[:], 1.0)
            nc.any.memset(Vp[:, :, 64:65], 1.0)
            nc.any.memset(Vp[:, :, 129:130], 1.0)

            # (weight staging happens inside the xstg pool below)

            # per-q-block AllToAll buffers over the INTERLEAVED row shard:
            # core j owns rows {512*n + 64*j + i}; chunk j of a2a_in[n] is
            # this core's 2 heads of that 64-row slice, so collective n can
            # fire right after q-block n's normalize.
            a2a_in = [dram.tile([4, 2, 64, 128], bf16, name=f"a2ain_{n}")
                      for n in range(nQB)]
            a2a_out = [dram.tile([8, 64, 128], bf16, name=f"a2aout_{n}")
                       for n in range(nQB)]

            # ---- phases 0-2, braided emission ------------------------
            # Engines execute their scheduled streams in static order, so
            # overlap must be built into emission order: the prep work
            # (x-load/transpose/qkv/V') for block n+1 is interleaved chunk-
            # by-chunk between the attention groups of q-block n.  Attention
            # qb=n depends only on qkv blocks 0..n, so each braid is legal.
            # PSUM banks: pA 2x1 + sT 2x2 + yt0 1 + yt1 1 = 8
            with (
                tc.tile_pool(name="xp", bufs=1) as xp,
                tc.tile_pool(name="xstg", bufs=3) as xstg,
                tc.tile_pool(name="ps", bufs=2, space="PSUM") as ps,
            ):
                xT = xp.tile([128, 8, seq], bf16)   # [c-chunk part, chunk, T]

                def wqkv_stage():
                    for k in range(8):
                        nc.sync.dma_start(wqkv[:, k, :],
                                          wq_d[128 * k:128 * (k + 1), :])

                def prep_chunks(n):
                    """Emit-closures for block n: loads, x^T xbar, qkv^T, V'."""
                    state = {}

                    def loads():
                        # x ships host-cast to bf16: DMA straight into the
                        # transpose staging tiles, no on-chip cast
                        xbs = []
                        for u in range(4):
                            t = 4 * n + u
                            xb = xstg.tile([128, D_MODEL], bf16, tag="xb",
                                           bufs=8, name=f"xb_{t}")
                            nc.sync.dma_start(xb[:],
                                              x_d[128 * t:128 * (t + 1), :])
                            xbs.append(xb)
                        state["xbs"] = xbs

                    def trans(j):
                        # j indexes (x-tile u = j//2, c-chunk quad a = j%2):
                        # one PSUM tile holds 4 c-chunk transposes of a
                        # single x-tile, so work starts after its one load
                        def emit():
                            u, a = divmod(j, 2)
                            tp = ps.tile([128, 512], f32, tag="pA",
                                         name=f"tp_{n}_{j}")
                            for c in range(4):
                                nc.tensor.matmul(
                                    tp[:, 128 * c:128 * (c + 1)],
                                    state["xbs"][u][:, 128 * (4 * a + c):
                                                    128 * (4 * a + c + 1)],
                                    iden[:], start=True, stop=True)
                            nc.vector.tensor_copy(
                                xT[:, 4 * a:4 * (a + 1),
                                   128 * (4 * n + u):
                                   128 * (4 * n + u + 1)],
                                tp[:])
                        return emit

                    def qkv(m):
                        def emit():
                            qp = ps.tile([128, 512], f32, tag="pA",
                                         name=f"qp_{n}_{m}")
                            for k in range(8):
                                nc.tensor.matmul(
                                    qp[:],
                                    wqkv[:, k, 128 * m:128 * (m + 1)],
                                    xT[:, k, 512 * n:512 * (n + 1)],
                                    start=(k == 0), stop=(k == 7))
                            if m == 0:
                                nc.vector.tensor_copy(
                                    qT[:, 512 * n:512 * (n + 1)], qp[:])
                            elif m == 1:
                                nc.vector.tensor_copy(
                                    kT[:, 512 * n:512 * (n + 1)], qp[:])
                            else:
                                vs = xstg.tile([128, 512], bf16, tag="vs",
                                               bufs=2, name=f"vs_{n}")
                                nc.vector.tensor_copy(vs[:], qp[:])
                                state["vs"] = vs
                        return emit

                    def vtr(u):
                        def emit():
                            t = 4 * n + u
                            vs = state["vs"]
                            # separate PSUM tiles per head: PE-write plus
                            # DVE-read of one PSUM bank is a HW fault
                            vp0 = ps.tile([128, 64], f32, tag="pA",
                                          name=f"vp0_{t}")
                            vp1 = ps.tile([128, 64], f32, tag="pA",
                                          name=f"vp1_{t}")
                            nc.tensor.matmul(
                                vp0[:], vs[0:64, 128 * u:128 * (u + 1)],
                                iden[0:64, 0:64], start=True, stop=True)
                            nc.tensor.matmul(
                                vp1[:], vs[64:128, 128 * u:128 * (u + 1)],
                                iden[64:128, 64:128], start=True, stop=True)
                            nc.vector.tensor_copy(Vp[:, t, 0:64], vp0[:])
                            nc.vector.tensor_copy(Vp[:, t, 65:129], vp1[:])
                        return emit

                    return ([loads] + [trans(j) for j in range(8)]
                            + [qkv(m) for m in range(3)]
                            + [vtr(u) for u in range(4)])

                carry = {}

                def off_of(qb, kt):
                    d = kt - 4 * qb
                    return 128 * d if d >= 0 else 0

                def emit_scores(qb, g):
                    # h-inner MM order: consecutive score matmuls use
                    # disjoint PE row-groups (h0 rows 0-63, h1 rows 64-127)
                    # so the 16x32x32-subarray PE overlaps them
                    sps = [ps.tile([128, 2 * QB], f32, tag="sT",
                                   name=f"sp_{qb}_{g}_{h}")
                           for h in (0, 1)]
                    for u in (0, 1):
                        kt = 2 * g + u
                        o = off_of(qb, kt)
                        for h in (0, 1):
                            nc.tensor.matmul(
                                sps[h][:, QB * u + o:QB * (u + 1)],
                                kT[64 * h:64 * (h + 1),
                                   128 * kt:128 * (kt + 1)],
                                qT[64 * h:64 * (h + 1),
                                   QB * qb + o:QB * (qb + 1)],
                                start=True, stop=True)
                    return sps

                def hoist_scores(qb):
                    # pre-emit the NEXT block's first score matmuls inside
                    # the current block's last-group filler, so the exp
                    # stream crosses the block boundary without a bubble
                    def emit():
                        carry[qb] = emit_scores(qb, 0)
                    return emit

                def attention_groups(qb, ytps, fill=None):
                    nkt = 4 * (qb + 1)

                    def group(g):
                        # diagonal k-tiles (d = kt-4qb >= 0) only attend to
                        # q >= 128d: trim score MM / exp / mask / yT MM to
                        # the valid column range [128d, QB).  q-cols below
                        # that are fully masked and, because kt=0 always
                        # covers the full width with start=True, never read.
                        def off(kt):
                            return off_of(qb, kt)

                        def emit():
                            if g == 0 and qb in carry:
                                sps = carry.pop(qb)
                            else:
                                sps = emit_scores(qb, g)
                            if fill is not None:
                                fill(g)
                            diag = off(2 * g) > 0 or off(2 * g + 1) > 0
                            for h in (0, 1):
                                pt = stg.tile([128, 2 * QB], bf16, tag="pT",
                                              bufs=3, name=f"pt_{qb}_{g}_{h}")
                                if diag:
                                    for u in (0, 1):
                                        o = off(2 * g + u)
                                        nc.scalar.activation(
                                            pt[:, QB * u + o:QB * (u + 1)],
                                            sps[h][:, QB * u + o:QB * (u + 1)],
                                            AFT.Exp, scale=0.125)
                                else:
                                    nc.scalar.activation(pt[:], sps[h][:],
                                                         AFT.Exp, scale=0.125)
                                for u in (0, 1):
                                    kt = 2 * g + u
                                    d = kt - 4 * qb
                                    o = off(kt)
                                    if d >= 0:
                                        w = min(o + 128, QB)
                                        nc.vector.tensor_mul(
                                            pt[:, QB * u + o:QB * u + w],
                                            pt[:, QB * u + o:QB * u + w],
                                            mks[:, d, o:w])
                                    # PV in [q, d] orientation: N=65 per
                                    # accumulation step instead of N=512
                                    # (matmul cost is out-free-size cycles).
                                    # One accumulation group per PSUM bank:
                                    # start marks the whole 2KB zero-region
                                    # pending-zero (first touch of each s
                                    # slot overwrites, later ones accumulate)
                                    for s in range(max(0, d), 4):
                                        nc.tensor.matmul(
                                            ytps[h][:, s, :],
                                            pt[:, QB * u + 128 * s:
                                               QB * u + 128 * (s + 1)],
                                            Vp[:, kt,
                                               65 * h:65 * (h + 1)],
                                            start=(kt == 0 and s == 0),
                                            stop=(kt == nkt - 1))
                        return emit

                    return [group(g) for g in range(nkt // 2)]

                def normalize(qb, ytps):
                    # y is [q, d] with q on partitions, so 1/denom is a
                    # per-partition scalar multiply -- no broadcast needed
                    ysn = stg.tile([128, 4, 128], bf16, tag="ysn", bufs=2,
                                   name=f"ysn_{qb}")
                    for h in (0, 1):
                        yq = stg.tile([128, 4, 66], f32, tag="dn", bufs=4,
                                      name=f"yq_{qb}_{h}")
                        nc.vector.tensor_copy(yq[:, :, 0:65], ytps[h][:])
                        nc.vector.reciprocal(yq[:, :, 65:66],
                                             yq[:, :, 64:65])
                        for s in range(4):
                            nc.vector.tensor_scalar_mul(
                                ysn[:, s, 64 * h:64 * (h + 1)],
                                yq[:, s, 0:64], yq[:, s, 65:66])
                    # stage this q-block's interleaved AllToAll rows:
                    # chunk j = (qsub j//2, partitions 64*(j%2)..) holds
                    # global rows {512*qb + 64*j + i} as [64 q, 128 c]
                    for jj in (0, 1):
                        nc.sync.dma_start(
                            a2a_in[qb][:, jj].rearrange("s p c -> p s c"),
                            ysn[64 * jj:64 * (jj + 1), :, :])

                def wpj_chunk(k):
                    def emit():
                        # w_proj ships host-cast to bf16: direct DMA into
                        # the weight tile, no staging or cast
                        nc.sync.dma_start(wpj[:, k, :],
                                          wp_d[128 * k:128 * (k + 1), :])
                    return emit

                def collective(qb):
                    def emit():
                        if use_collective:
                            nc.gpsimd.collective_compute(
                                "AllToAll", mybir.AluOpType.bypass,
                                ins=[a2a_in[qb].opt()],
                                outs=[a2a_out[qb].opt()],
                                replica_groups=[list(range(N_CORES))])
                        else:
                            # timing-model stand-in (TimelineSim can't
                            # execute collectives): DRAM->DRAM copy
                            nc.sync.dma_start(a2a_out[qb].opt(),
                                              a2a_in[qb].opt())
                    return emit

                def rx_piece(p):
                    """Projection for the 128 interleaved rows of q-blocks
                    2p, 2p+1 (this core's shard of those blocks); braided
                    into a late attention block whose collectives are done."""
                    st = {}

                    def rx_loads():
                        rxq = stg.tile([64, 2, 8, 128], bf16, tag="rxq",
                                       bufs=2, name=f"rxq_{p}")
                        for e in (0, 1):
                            nc.sync.dma_start(
                                rxq[:, e],
                                a2a_out[2 * p + e].rearrange("i p c -> p i c"))
                        st["rxq"] = rxq

                    def rtr():
                        # received rows are [q, c]; transpose per 128-c chunk
                        # into the projection's lhsT layout [c, q]
                        rxT = stg.tile([128, 8, 128], bf16, tag="rx",
                                       bufs=2, name=f"rxT_{p}")
                        for e in (0, 1):
                            ytp = ps.tile([128, 8, 64], f32, tag="pA",
                                          name=f"ytp_{p}_{e}")
                            for i in range(8):
                                nc.tensor.matmul(
                                    ytp[:, i, :], st["rxq"][:, e, i, :],
                                    iden[0:64, 0:64], start=True, stop=True)
                            nc.vector.tensor_copy(
                                rxT[:, :, 64 * e:64 * (e + 1)], ytp[:])
                        st["rx"] = rxT

                    def half(n2):
                        def emit():
                            pp = ps.tile([128, 512], f32, tag="pA",
                                         name=f"pp_{p}_{n2}")
                            for k in range(8):
                                nc.tensor.matmul(
                                    pp[:],
                                    st["rx"][:, k, :],
                                    wpj[:, k, 512 * n2:512 * (n2 + 1)],
                                    start=(k == 0), stop=(k == 7))
                            ob = stg.tile([128, 512], f32, tag="ob", bufs=2,
                                          name=f"ob_{p}_{n2}")
                            nc.vector.tensor_copy(ob[:], pp[:])
                            nc.sync.dma_start(
                                out_d[128 * p:128 * (p + 1),
                                      512 * n2:512 * (n2 + 1)], ob[:])
                        return emit

                    return [rx_loads, rtr, half(0), half(1)]

                p0 = prep_chunks(0)
                p0[0]()           # stage-0 x loads lead the DMA queues
                wqkv_stage()
                for m in range(4):
                    # masks load after the startup-critical x/w DMAs
                    nc.scalar.dma_start(mks[:, m, :], mk_d[m])
                for c in p0[1:]:
                    c()
                for n in range(nQB):
                    ytps = [ps.tile([128, 4, 65], f32, tag=f"yt{h}", bufs=1,
                                    name=f"yt{h}_{n}") for h in (0, 1)]
                    pend = prep_chunks(n + 1) if n + 1 < nQB else []
                    if n >= 1:
                        # issue q-block n-1's AllToAll one block late so its
                        # wait (on the staging DMAs) is already satisfied and
                        # never head-of-line blocks the Pool queue
                        pend = pend[:1] + [collective(n - 1)] + pend[1:]
                    if 3 <= n <= 4:
                        # stage w_proj in blocks 3-4: late enough to stay
                        # clear of the prep-limited early blocks, done
                        # before piece 0 consumes it at block 5
                        ks = {3: (0, 1, 2, 3), 4: (4, 5, 6, 7)}[n]
                        pend = pend + [wpj_chunk(k) for k in ks]
                    if n >= 5:
                        # piece p covers q-blocks 2p, 2p+1; its collectives
                        # completed >= 2 blocks ago
                        pend = pend + rx_piece(n - 5)
                    if n + 1 < nQB:
                        pend = pend + [hoist_scores(n + 1)]
                    st = {"ci": 0}

                    def fill(gi):
                        want = (gi + 1) * len(pend) // (2 * (n + 1))
                        while st["ci"] < want:
                            pend[st["ci"]]()
                            st["ci"] += 1

                    groups = attention_groups(n, ytps, fill)
                    for g in groups:
                        g()
                    while st["ci"] < len(pend):
                        pend[st["ci"]]()
                        st["ci"] += 1
                    normalize(n, ytps)
                collective(nQB - 1)()

            # ---- tail: projection piece for q-blocks 6,7 ------------------
            with tc.tile_pool(name="psC", bufs=1, space="PSUM") as psC:
                rxq = stg.tile([64, 2, 8, 128], bf16, tag="rxq", bufs=2,
                               name="rxq_3")
                for e in (0, 1):
                    nc.sync.dma_start(
                        rxq[:, e],
                        a2a_out[6 + e].rearrange("i p c -> p i c"))
                rx = stg.tile([128, 8, 128], bf16, tag="rx", bufs=2,
                              name="rxT_3")
                for e in (0, 1):
                    ytp = psC.tile([128, 8, 64], f32, tag="ytpC",
                                   name=f"ytpC_{e}")
                    for i in range(8):
                        nc.tensor.matmul(
                            ytp[:, i, :], rxq[:, e, i, :],
                            iden[0:64, 0:64], start=True, stop=True)
                    nc.vector.tensor_copy(
                        rx[:, :, 64 * e:64 * (e + 1)], ytp[:])
                for n2 in (0, 1):
                    pp = psC.tile([128, 512], f32, tag="ppC",
                                  name=f"ppC_{n2}")
                    for k in range(8):
                        nc.tensor.matmul(
                            pp[:], rx[:, k, :],
                            wpj[:, k, 512 * n2:512 * (n2 + 1)],
                            start=(k == 0), stop=(k == 7))
                    ob = stg.tile([128, 512], f32, tag="ob", bufs=2,
                                  name=f"obC_{n2}")
                    nc.vector.tensor_copy(ob[:], pp[:])
                    nc.sync.dma_start(
                        out_d[384:512, 512 * n2:512 * (n2 + 1)], ob[:])

    if split_waits:
        _split_multi_waits(nc)
    return nc


def make_aux_inputs():
    ident = np.eye(128, dtype=BF16)
    k_idx = np.arange(128)[:, None]
    q_idx = np.arange(QB)[None, :]
    masks = np.stack(
        [((k_idx + 128 * d) <= q_idx).astype(BF16) for d in range(4)], axis=0)
    return ident, masks


def make_in_maps(x, w_qkv, w_proj, seq=SEQ):
    x = np.asarray(x, dtype=np.float32).reshape(seq, D_MODEL).astype(BF16)
    w_qkv = np.asarray(w_qkv, dtype=np.float32)
    w_proj = np.asarray(w_proj, dtype=np.float32).astype(BF16)
    ident, masks = make_aux_inputs()
    in_maps = []
    for i in range(N_CORES):
        sl = slice(CPC * i, CPC * (i + 1))
        w_slice = np.concatenate(
            [w_qkv[:, sl], w_qkv[:, D_MODEL:][:, sl],
             w_qkv[:, 2 * D_MODEL:][:, sl]], axis=1)
        in_maps.append({
            "x": x,
            "w_slice": np.ascontiguousarray(w_slice).astype(BF16),
            "w_proj": w_proj,
            "ident": ident,
            "masks": masks,
        })
    return in_maps


_NC_CACHE = {}


def kernel(x, w_qkv, w_proj):
    """Full inputs in, full output out. Shards internally across 8 cores."""
    try:
        import os
        import jax
        jax.config.update("jax_compilation_cache_dir",
                          os.path.expanduser("~/.cache/jax_bass_kernel"))
        jax.config.update("jax_persistent_cache_min_compile_time_secs", 0.0)
    except Exception:
        pass
    from concourse.bass_utils import run_bass_kernel_spmd

    x = np.asarray(x, dtype=np.float32)
    batch = x.shape[0]
    seq = x.shape[1]
    if seq not in _NC_CACHE:
        _NC_CACHE[seq] = build_nc(seq)
    nc = _NC_CACHE[seq]
    in_maps = make_in_maps(x, w_qkv, w_proj, seq=seq)
    res = run_bass_kernel_spmd(nc, in_maps, list(range(N_CORES)))
    # core j's rows are interleaved: local row 64*n + i (n = q-block) is
    # global row 512*n + 64*j + i
    out = np.empty((seq, D_MODEL), dtype=np.float32)
    outr = out.reshape(seq // QB, N_CORES, 64, D_MODEL)
    for j in range(N_CORES):
        outr[:, j] = res.results[j]["out"].reshape(seq // QB, 64, D_MODEL)
    return out.reshape(batch, seq, D_MODEL).astype(np.float32)

